# revision 49
# baseline (speedup 1.0000x reference)
"""Trainium2 Bass kernel for an AttentionBlock (GroupNorm + MHA + residual).

Shapes (hardcoded from the problem spec):
  x: [8, 32, 32, 512] fp32, t: [8] (unused by the reference),
  gn_scale/gn_bias: [512], w_qkv: [512, 1536], b_qkv: [1536],
  w_out: [512, 512], b_out: [512].

Sharding: pure data parallel — one batch element per NeuronCore (8 cores).
Each core runs the full block on its [1024, 512] slice; no collectives.

Per-core compute layout:
  - GroupNorm stats via ones-vector matmuls (sum over s on the PE),
    rsqrt as exp(-0.5*ln(var+eps)) so ScalarE stays in one table set.
  - xn transposed to [c, s] with PE transposes; qT/kT computed per head-pair
    [128, 1024] so QK^T row-packs two K=64 heads onto the PE array.
  - scores are computed transposed ([sk, sq]) so the softmax denominator and
    the attention*V contraction both come out of matmuls without transposing
    the [1024, 1024] attention matrix: AV uses v_aug=[v | 1] (M=65) stationary,
    and row 64 of the PSUM accumulator is the softmax row-sum.
  - exp runs on ScalarE straight out of PSUM, bf16 out.
  - matmul operands are bf16 (fp32 matmuls are 4x slower on TRN2); all
    accumulation is fp32 in PSUM; GroupNorm/normalize/residual math is fp32.

PSUM budget (8 banks, statically reserved by tile tags):
  psc [128,1024] f32 x1 = 2 banks (also used for the GN stats accumulator),
  acc [128,512] f32 x2 = 2 banks (transposes, qkv/out-proj accumulators),
  po  [128,512] f32 x4 = 4 banks (per-pair AV accumulators, 2 heads x 2 chunks).
"""

import numpy as np
import ml_dtypes

import concourse.bass as bass
import concourse.mybir as mybir
import concourse.tile as tile
from concourse.masks import make_identity


def _legalize_bir(bir_json: bytes) -> bytes:
    """This container's walrus encodes at most ONE sync wait per instruction
    ('Too many sync wait commands' otherwise), while the Tile scheduler freely
    emits 2+ (and the tail drain waits on every live semaphore). Legalize at
    the BIR level: for every instruction with N>1 waits, insert N-1 NoOps
    carrying one wait each immediately before it on the same engine — the
    engine executes them serially, so blocking semantics are identical."""
    import json

    m = json.loads(bir_json)
    n = 0
    for fn in m["functions"]:
        for bb in fn["blocks"]:
            out = []
            for inst in bb["instructions"]:
                si = inst.get("sync_info")
                waits = (si or {}).get("on_wait", [])
                if si is not None and len(waits) > 1:
                    for w in waits[:-1]:
                        n += 1
                        out.append({
                            "debug": inst.get("debug", 0),
                            "engine": inst["engine"],
                            "ins": [],
                            "outs": [],
                            "name": f"I-sqw-{n}",
                            "opcode": "NoOp",
                            "sync_info": {"on_update": [], "on_wait": [w]},
                        })
                    si["on_wait"] = [waits[-1]]
                out.append(inst)
            bb["instructions"] = out
    return json.dumps(m).encode()


def _install_bir_legalizer():
    import concourse.bass_utils as bu
    import concourse.bass2jax as b2j

    if getattr(bu, "_bir_legalizer_installed", False):
        return
    orig = bu.compile_bir_kernel

    def wrapped(bir_json, tmpdir, neff_name="file.neff"):
        return orig(_legalize_bir(bir_json), tmpdir, neff_name)

    bu.compile_bir_kernel = wrapped
    b2j.compile_bir_kernel = wrapped
    bu._bir_legalizer_installed = True


FP32 = mybir.dt.float32
BF16 = mybir.dt.bfloat16
AF = mybir.ActivationFunctionType
ALU = mybir.AluOpType

B = 8
S = 1024          # 32 * 32
C = 512
NH = 8
HD = 64
INNER = NH * HD   # 512
G = 32            # groups
GS = C // G       # 16 channels per group
EPS = 1e-5
N_CORES = 8

ST = S // 128     # 8 s-tiles
CT = C // 128     # 4 c-tiles
PAIRS = NH // 2   # 4 head pairs
SQC = S // 512    # 2 sq chunks of 512


def _pack_weights(wqkv_f32: np.ndarray, wout_f32: np.ndarray):
    """Host-side packing: w_qkv [C, 3*INNER] -> (w_qk [C, 1024], w_v [C, 512])
    so every matmul stationary slice is contiguous (1 free dim).

    w_qk layout: [c, pair, {q,k}, head_in_pair, d] flattened;
    w_v layout:  [c, head, d] flattened."""
    bf = ml_dtypes.bfloat16
    r = wqkv_f32.reshape(C, NH, 3, HD)
    wqk = (
        r[:, :, 0:2, :]
        .reshape(C, PAIRS, 2, 2, HD)  # [c, pair, h2, j, d]
        .transpose(0, 1, 3, 2, 4)     # [c, pair, j, h2, d]
        .reshape(C, PAIRS * 2 * 2 * HD)
        .astype(bf)
    )
    wv = r[:, :, 2, :].reshape(C, INNER).astype(bf)
    return np.ascontiguousarray(wqk), np.ascontiguousarray(wv), wout_f32.astype(bf)


def _dma(nc, out, in_):
    nc.sync.dma_start(out, in_)


def _emit(nc: bass.Bass, has_bqk: bool, has_bv: bool, has_bout: bool):
    x_d = nc.dram_tensor("x", [S, C], FP32, kind="ExternalInput")
    wqk_d = nc.dram_tensor("w_qk", [C, PAIRS * 2 * 128], BF16, kind="ExternalInput")
    wv_d = nc.dram_tensor("w_v", [C, INNER], BF16, kind="ExternalInput")
    wout_d = nc.dram_tensor("w_out", [INNER, C], BF16, kind="ExternalInput")
    bqkv_d = nc.dram_tensor("b_qkv", [3 * INNER], FP32, kind="ExternalInput")
    bout_d = nc.dram_tensor("b_out", [C], FP32, kind="ExternalInput")
    gns_d = nc.dram_tensor("gn_scale", [C], FP32, kind="ExternalInput")
    gnb_d = nc.dram_tensor("gn_bias", [C], FP32, kind="ExternalInput")
    y_d = nc.dram_tensor("y", [S, C], FP32, kind="ExternalOutput")
    gn_scratch = nc.dram_tensor("gn_scratch", [2, C], FP32, kind="Internal")

    with tile.TileContext(nc) as tc:
        with (
            tc.tile_pool(name="const", bufs=1) as const,
            tc.tile_pool(name="big", bufs=1) as big,
            tc.tile_pool(name="work", bufs=2) as work,
            tc.tile_pool(name="expp", bufs=3) as expp,
            tc.tile_pool(name="ps", bufs=1, space="PSUM") as ps,
        ):
            # ---------- constants / weights ----------
            ident = const.tile([128, 128], BF16)
            make_identity(nc, ident)
            ones_col = const.tile([128, 1], BF16)
            nc.vector.memset(ones_col, 1.0)
            eps_t = const.tile([1, 1], FP32)
            nc.vector.memset(eps_t, EPS)
            actwarm = const.tile([1, 2], FP32)
            nc.scalar.activation(actwarm[:, 0:1], eps_t, AF.Ln)
            nc.scalar.activation(actwarm[:, 1:2], eps_t, AF.Exp)

            # ---------- load x (per-tile DMAs for pipelining) ----------
            x_sb = big.tile([128, ST, C], FP32)
            x_t = x_d.rearrange("(t p) c -> t p c", p=128)
            for t in range(ST):
                _dma(nc, x_sb[:, t], x_t[t])

            wqk_sb = const.tile([128, CT, PAIRS, 2, 128], BF16)
            _dma(nc, 
                wqk_sb,
                wqk_d.rearrange("(ct p) (pr j f) -> p ct pr j f", p=128, pr=PAIRS, j=2),
            )
            wv_sb = const.tile([128, CT, INNER], BF16)
            _dma(nc, wv_sb, wv_d.rearrange("(ct p) f -> p ct f", p=128))
            wout_sb = const.tile([128, CT, C], BF16)
            _dma(nc, wout_sb, wout_d.rearrange("(ct p) f -> p ct f", p=128))
            gamma = const.tile([1, G, GS], FP32)
            _dma(nc, gamma, gns_d[None, :])
            beta = const.tile([1, G, GS], FP32)
            _dma(nc, beta, gnb_d[None, :])

            bqkv_hx = bqkv_d[None, :].rearrange("o (h x) -> o h x", x=3 * HD)
            bqk = bv_bc = bout_bc = None
            if has_bqk:
                # per-partition bias columns for the q/k projections per pair
                bqk = const.tile([128, PAIRS, 2], FP32)
                for p in range(PAIRS):
                    for j in range(2):  # 0 -> q cols, 1 -> k cols
                        _dma(nc,
                            bqk[:, p, j : j + 1],
                            bqkv_hx[0, 2 * p : 2 * p + 2, j * HD : (j + 1) * HD],
                        )
            if has_bv:
                # free-dim bias vector broadcast to 128 partitions via DMA
                # (DRAM-side partition broadcast is allowed, SBUF-side is not)
                bv_bc = const.tile([128, NH, HD], FP32)
                _dma(nc,
                    bv_bc, bqkv_hx[:, :, 2 * HD : 3 * HD].partition_broadcast(128)
                )
            if has_bout:
                bout_bc = const.tile([128, C], FP32)
                _dma(nc, bout_bc, bout_d[None, :].partition_broadcast(128))

            # ---------- GroupNorm stats (the critical chain: emit first so the
            # stats -> scale/bias pipeline outranks the transpose evacuations)
            xbf = big.tile([128, ST, C], BF16)
            xT = [
                big.tile([128, S], BF16, tag=f"xT{ct}", name=f"xT{ct}")
                for ct in range(CT)
            ]
            pstat = ps.tile([1, 2, C], FP32, tag="big2", bufs=3)
            for t in range(ST):
                nc.gpsimd.tensor_copy(xbf[:, t], x_sb[:, t])
                xsq_t = work.tile([128, C], BF16, tag="xsq")
                nc.gpsimd.tensor_mul(xsq_t, xbf[:, t], xbf[:, t])
                nc.tensor.matmul(
                    pstat[:, 0], ones_col, xbf[:, t], start=(t == 0), stop=(t == ST - 1)
                )
                nc.tensor.matmul(
                    pstat[:, 1], ones_col, xsq_t, start=(t == 0), stop=(t == ST - 1)
                )

            gstat = work.tile([1, 2, G], FP32, tag="gstat")
            nc.vector.reduce_sum(
                gstat,
                pstat.rearrange("p s (g d) -> p s g d", d=GS),
                axis=mybir.AxisListType.X,
            )
            inv_n = 1.0 / (S * GS)
            mean_g = work.tile([1, G], FP32, tag="mean_g")
            nc.vector.tensor_scalar_mul(mean_g, gstat[:, 0], inv_n)
            var_g = work.tile([1, G], FP32, tag="var_g")
            nc.vector.tensor_scalar_mul(var_g, gstat[:, 1], inv_n)
            msq = work.tile([1, G], FP32, tag="msq")
            nc.vector.tensor_mul(msq, mean_g, mean_g)
            nc.vector.tensor_sub(var_g, var_g, msq)
            # rstd = exp(-0.5 * ln(var + eps))  (stays in the exp table set)
            lnv = work.tile([1, G], FP32, tag="lnv")
            nc.scalar.activation(lnv, var_g, AF.Ln, bias=eps_t)
            rstd_g = work.tile([1, G], FP32, tag="rstd_g")
            nc.scalar.activation(rstd_g, lnv, AF.Exp, scale=-0.5)

            # sb_c[0] = scale_c = rstd_g * gamma ; sb_c[1] = bias_c
            sb_c = work.tile([1, 2, G, GS], FP32, tag="sb_c")
            scale_c = sb_c[:, 0]
            bias_c = sb_c[:, 1]
            nc.vector.tensor_tensor(
                scale_c, gamma, rstd_g[:, :, None].to_broadcast([1, G, GS]), ALU.mult
            )
            nc.vector.tensor_tensor(
                bias_c, scale_c, mean_g[:, :, None].to_broadcast([1, G, GS]), ALU.mult
            )
            nc.vector.tensor_sub(bias_c, beta, bias_c)

            # per-partition (channel-major) copies of scale/bias for the
            # transposed-side GN application on ScalarE (DRAM bounce: DMA
            # cannot scatter SBUF free dim -> partitions directly)
            nc.gpsimd.dma_start(
                gn_scratch[None, :], sb_c.rearrange("o s g d -> o s (g d)")
            )
            sbp = const.tile([128, 2, CT], FP32)
            nc.gpsimd.dma_start(sbp, gn_scratch.rearrange("s (ct p) -> p s ct", p=128))
            scale_cp = sbp[:, 0]
            bias_cp = sbp[:, 1]
            # broadcast copies for the natural-layout residual xn
            scale_bc = const.tile([128, C], FP32)
            _dma(nc,
                scale_bc,
                scale_c[:, None, :, :].to_broadcast([1, 128, G, GS]),
            )
            bias_bc = const.tile([128, C], FP32)
            _dma(nc,
                bias_bc,
                bias_c[:, None, :, :].to_broadcast([1, 128, G, GS]),
            )

            # transposes of raw x (independent of the stats chain, lower prio)
            for t in range(ST):
                for ct in range(CT):
                    ptr = ps.tile([128, 128], BF16, tag="acc", bufs=2)
                    nc.tensor.transpose(
                        ptr, xbf[:, t, 128 * ct : 128 * (ct + 1)], ident
                    )
                    nc.vector.tensor_copy(xT[ct][:, 128 * t : 128 * (t + 1)], ptr)

            # ---------- GN on the transposed side: xnT = xT*scale + bias ----
            xnT = [
                big.tile([128, S], BF16, tag=f"xnT{ct}", name=f"xnT{ct}")
                for ct in range(CT)
            ]
            for ct in range(CT):
                nc.scalar.activation(
                    xnT[ct],
                    xT[ct],
                    AF.Identity,
                    bias=bias_cp[:, ct : ct + 1],
                    scale=scale_cp[:, ct : ct + 1],
                )

            # ---------- per head pair: q/k projections, then attention ----------
            # outT is split per (pair, sq-chunk) so the output projection can
            # start as soon as the chunk it reads is normalized.
            outT = [
                [
                    big.tile(
                        [128, 512], BF16, tag=f"outT{p}_{cc}", name=f"outT{p}_{cc}"
                    )
                    for cc in range(SQC)
                ]
                for p in range(PAIRS)
            ]

            def alloc_qk(p):
                qT = work.tile([128, S], BF16, tag="qT", name=f"qT{p}")
                kT = work.tile([128, S], BF16, tag="kT", name=f"kT{p}")
                return qT, kT

            def emit_qk_group(p, qT, kT, j, cchunk):
                dst = (qT, kT)[j]
                pqk = ps.tile([128, 512], FP32, tag="acc", bufs=2)
                for ct in range(CT):
                    nc.tensor.matmul(
                        pqk,
                        wqk_sb[:, ct, p, j],
                        xnT[ct][:, 512 * cchunk : 512 * (cchunk + 1)],
                        start=(ct == 0),
                        stop=(ct == CT - 1),
                    )
                if has_bqk:
                    nc.vector.tensor_scalar(
                        dst[:, 512 * cchunk : 512 * (cchunk + 1)],
                        pqk,
                        bqk[:, p, j : j + 1],
                        None,
                        ALU.add,
                    )
                else:
                    nc.vector.tensor_copy(
                        dst[:, 512 * cchunk : 512 * (cchunk + 1)], pqk
                    )

            qk_tiles = {0: alloc_qk(0)}
            for j in (0, 1):
                for cc in range(SQC):
                    emit_qk_group(0, *qk_tiles[0], j, cc)

            # ---------- v projection (natural layout) + ones column ----------
            # emitted lazily inside pair-0/head-0's loop so the first QK
            # matmuls outrank it in the PE's static order
            v_sb = big.tile([128, ST, NH, HD + 1], BF16)

            def emit_v(t):
                pv = ps.tile([128, INNER], FP32, tag="acc", bufs=2)
                for ct in range(CT):
                    nc.tensor.matmul(
                        pv,
                        xnT[ct][:, 128 * t : 128 * (t + 1)],
                        wv_sb[:, ct],
                        start=(ct == 0),
                        stop=(ct == CT - 1),
                    )
                if has_bv:
                    nc.vector.tensor_tensor(
                        v_sb[:, t, :, 0:HD],
                        pv.rearrange("p (h d) -> p h d", d=HD),
                        bv_bc,
                        ALU.add,
                    )
                else:
                    nc.vector.tensor_copy(
                        v_sb[:, t, :, 0:HD], pv.rearrange("p (h d) -> p h d", d=HD)
                    )
                nc.gpsimd.memset(v_sb[:, t, :, HD : HD + 1], 1.0)

            # ---------- natural-layout xn for the residual (off critical path)
            xn = big.tile([128, ST, C], FP32)
            for t in range(ST):
                nc.vector.tensor_tensor(xn[:, t], x_sb[:, t], scale_bc, ALU.mult)
                nc.vector.tensor_tensor(xn[:, t], xn[:, t], bias_bc, ALU.add)

            for p in range(PAIRS):
                qT, kT = qk_tiles.pop(p)
                if p + 1 < PAIRS:
                    qk_tiles[p + 1] = alloc_qk(p + 1)
                    next_groups = [(j, cc) for j in (0, 1) for cc in range(SQC)]
                # attention, one head at a time. The PE is in-order, so AV
                # matmuls (which wait on exp) are emitted with a 2-step lag
                # behind the QK matmuls: the exp chain on ScalarE then never
                # waits for the PE.
                for h in range(2):
                    po_h = ps.tile(
                        [HD + 1, SQC, 512], FP32, tag="big2", bufs=3, name=f"po{p}_{h}"
                    )

                    def emit_av(skt, et):
                        for cc in range(SQC):
                            nc.tensor.matmul(
                                po_h[:, cc, :],
                                v_sb[:, skt, 2 * p + h],
                                et[:, 512 * cc : 512 * (cc + 1)],
                                start=(skt == 0),
                                stop=(skt == ST - 1),
                            )

                    pend = []
                    for skt in range(ST):
                        psc = ps.tile([128, S], FP32, tag="big2", bufs=3)
                        for cc in range(SQC):
                            nc.tensor.matmul(
                                psc[:, 512 * cc : 512 * (cc + 1)],
                                kT[64 * h : 64 * (h + 1), 128 * skt : 128 * (skt + 1)],
                                qT[64 * h : 64 * (h + 1), 512 * cc : 512 * (cc + 1)],
                                start=True,
                                stop=True,
                            )
                        et = expp.tile([128, S], BF16, tag="exp", bufs=12)
                        nc.scalar.activation(et, psc, AF.Exp, scale=1.0 / 8.0)
                        if p == 0 and h == 0:
                            emit_v(skt)
                        if h == 1 and p + 1 < PAIRS and skt % 2 == 0:
                            j, cc = next_groups[skt // 2]
                            emit_qk_group(p + 1, *qk_tiles[p + 1], j, cc)
                        pend.append((skt, et))
                        if len(pend) > 4:
                            emit_av(*pend.pop(0))
                    for item in pend:
                        emit_av(*item)
                    # normalize: out / rowsum (rowsum is PSUM row 64).
                    # 1/r = exp(-ln(r)) on ScalarE: a 1-lane DVE reciprocal is
                    # ~8 cycles/element (4us per row on HW) while ACT runs
                    # 1 cycle/element and reads PSUM directly. The fp32
                    # reciprocal row is then broadcast across the 64 head-dim
                    # partitions with a stride-0 SBUF->SBUF DMA.
                    lnr = work.tile([1, SQC, 512], FP32, tag="lnr")
                    nc.scalar.activation(lnr, po_h[HD : HD + 1, :, :], AF.Ln)
                    recip = work.tile([1, SQC, 512], FP32, tag="recip")
                    nc.scalar.activation(recip, lnr, AF.Exp, scale=-1.0)
                    pou = work.tile([HD, SQC, 512], FP32, tag="pou")
                    nc.vector.tensor_copy(pou, po_h[0:HD, :, :])
                    for cc in range(SQC):
                        rbc = work.tile([HD, 512], FP32, tag="rbc")
                        nc.gpsimd.dma_start(
                            rbc,
                            recip[:, cc][:, None, :].to_broadcast([1, HD, 512]),
                        )
                        nc.vector.tensor_tensor(
                            outT[p][cc][64 * h : 64 * (h + 1), :],
                            pou[:, cc, :],
                            rbc,
                            ALU.mult,
                        )

            # ---------- output projection + residual ----------
            for t in range(ST):
                ph = ps.tile([128, C], FP32, tag="acc", bufs=2)
                for p in range(PAIRS):
                    nc.tensor.matmul(
                        ph,
                        outT[p][t // 4][:, 128 * (t % 4) : 128 * (t % 4 + 1)],
                        wout_sb[:, p],
                        start=(p == 0),
                        stop=(p == PAIRS - 1),
                    )
                yt = work.tile([128, C], FP32, tag="y")
                nc.vector.tensor_tensor(yt, ph, xn[:, t], ALU.add)
                if has_bout:
                    nc.vector.tensor_tensor(yt, yt, bout_bc, ALU.add)
                _dma(nc, y_d[128 * t : 128 * (t + 1), :], yt)

    return nc


_CACHE: dict = {}


def _get_program(has_bqk: bool = True, has_bv: bool = True, has_bout: bool = True) -> bass.Bass:
    key = (has_bqk, has_bv, has_bout)
    if key not in _CACHE:
        _install_bir_legalizer()
        nc = bass.Bass()
        _emit(nc, has_bqk, has_bv, has_bout)
        _CACHE[key] = nc
    return _CACHE[key]


def _get_runner(nc: bass.Bass):
    """Build the sharded PJRT executable once and cache it — bass2jax's
    run_bass_via_pjrt re-jits a fresh closure every call (~1.5 s of host-side
    retrace per invocation)."""
    if "runner" in _CACHE and _CACHE["runner"][0] is nc:
        return _CACHE["runner"][1]

    import jax
    from jax.sharding import Mesh, PartitionSpec
    from jax.experimental.shard_map import shard_map
    from concourse import bass2jax, mybir as _mb

    bass2jax.install_neuronx_cc_hook()
    partition_name = (
        nc.partition_id_tensor.name if nc.partition_id_tensor else None
    )
    in_names, out_names, out_avals, zero_outs = [], [], [], []
    for alloc in nc.m.functions[0].allocations:
        if not isinstance(alloc, _mb.MemoryLocationSet):
            continue
        name = alloc.memorylocations[0].name
        if alloc.kind == "ExternalInput":
            if name != partition_name:
                in_names.append(name)
        elif alloc.kind == "ExternalOutput":
            shape = tuple(alloc.tensor_shape)
            dtype = _mb.dt.np(alloc.dtype)
            out_names.append(name)
            out_avals.append(jax.core.ShapedArray(shape, dtype))
            zero_outs.append(np.zeros(shape, dtype))
    n_params = len(in_names)
    n_outs = len(out_avals)
    all_in_names = list(in_names) + list(out_names)
    if partition_name is not None:
        all_in_names.append(partition_name)
    donate = tuple(range(n_params, n_params + n_outs))

    def _body(*args):
        operands = list(args)
        if partition_name is not None:
            operands.append(bass2jax.partition_id_tensor())
        outs = bass2jax._bass_exec_p.bind(
            *operands,
            out_avals=tuple(out_avals),
            in_names=tuple(all_in_names),
            out_names=tuple(out_names),
            lowering_input_output_aliases=(),
            sim_require_finite=True,
            sim_require_nnan=True,
            nc=nc,
        )
        return tuple(outs)

    devices = jax.devices()[:N_CORES]
    mesh = Mesh(np.asarray(devices), ("core",))
    in_specs = (PartitionSpec("core"),) * (n_params + n_outs)
    out_specs = (PartitionSpec("core"),) * n_outs
    sharded = jax.jit(
        shard_map(
            _body, mesh=mesh, in_specs=in_specs, out_specs=out_specs,
            check_rep=False,
        ),
        donate_argnums=donate,
        keep_unused=True,
    )

    def run(in_maps):
        per_core = [[np.asarray(m[name]) for name in in_names] for m in in_maps]
        concat_in = [
            np.concatenate([per_core[c][i] for c in range(N_CORES)], axis=0)
            for i in range(n_params)
        ]
        concat_zeros = [
            np.zeros((N_CORES * z.shape[0], *z.shape[1:]), z.dtype)
            for z in zero_outs
        ]
        out_arrs = sharded(*concat_in, *concat_zeros)
        return [
            {
                name: np.asarray(out_arrs[i]).reshape(
                    N_CORES, *out_avals[i].shape
                )[c]
                for i, name in enumerate(out_names)
            }
            for c in range(N_CORES)
        ]

    _CACHE["runner"] = (nc, run)
    return run


def kernel(**inputs) -> np.ndarray:
    x = np.asarray(inputs["x"], dtype=np.float32)
    wqk, wv, wout = _pack_weights(
        np.asarray(inputs["w_qkv"], dtype=np.float32),
        np.asarray(inputs["w_out"], dtype=np.float32),
    )
    common = {
        "w_qk": wqk,
        "w_v": wv,
        "w_out": wout,
        "b_qkv": np.asarray(inputs["b_qkv"], dtype=np.float32),
        "b_out": np.asarray(inputs["b_out"], dtype=np.float32),
        "gn_scale": np.asarray(inputs["gn_scale"], dtype=np.float32),
        "gn_bias": np.asarray(inputs["gn_bias"], dtype=np.float32),
    }
    in_maps = [
        {"x": np.ascontiguousarray(x[i].reshape(S, C)), **common} for i in range(B)
    ]
    bq = common["b_qkv"]
    nc = _get_program(
        has_bqk=bool(np.any(bq.reshape(NH, 3, HD)[:, 0:2, :])),
        has_bv=bool(np.any(bq.reshape(NH, 3, HD)[:, 2, :])),
        has_bout=bool(np.any(common["b_out"])),
    )
    res = _get_runner(nc)(in_maps)
    y = np.stack([r["y"] for r in res], axis=0)
    return y.reshape(B, 32, 32, C).astype(np.float32)


# revision 50
# speedup vs baseline: 1.0334x; 1.0334x over previous
"""Trainium2 Bass kernel for an AttentionBlock (GroupNorm + MHA + residual).

Shapes (hardcoded from the problem spec):
  x: [8, 32, 32, 512] fp32, t: [8] (unused by the reference),
  gn_scale/gn_bias: [512], w_qkv: [512, 1536], b_qkv: [1536],
  w_out: [512, 512], b_out: [512].

Sharding: pure data parallel — one batch element per NeuronCore (8 cores).
Each core runs the full block on its [1024, 512] slice; no collectives.

Per-core compute layout:
  - GroupNorm stats via ones-vector matmuls (sum over s on the PE),
    rsqrt as exp(-0.5*ln(var+eps)) so ScalarE stays in one table set.
  - xn transposed to [c, s] with PE transposes; qT/kT computed per head-pair
    [128, 1024] so QK^T row-packs two K=64 heads onto the PE array.
  - scores are computed transposed ([sk, sq]) so the softmax denominator and
    the attention*V contraction both come out of matmuls without transposing
    the [1024, 1024] attention matrix: AV uses v_aug=[v | 1] (M=65) stationary,
    and row 64 of the PSUM accumulator is the softmax row-sum.
  - exp runs on ScalarE straight out of PSUM, bf16 out.
  - matmul operands are bf16 (fp32 matmuls are 4x slower on TRN2); all
    accumulation is fp32 in PSUM; GroupNorm/normalize/residual math is fp32.

PSUM budget (8 banks, statically reserved by tile tags):
  psc [128,1024] f32 x1 = 2 banks (also used for the GN stats accumulator),
  acc [128,512] f32 x2 = 2 banks (transposes, qkv/out-proj accumulators),
  po  [128,512] f32 x4 = 4 banks (per-pair AV accumulators, 2 heads x 2 chunks).
"""

import numpy as np
import ml_dtypes

import concourse.bass as bass
import concourse.mybir as mybir
import concourse.tile as tile
from concourse.masks import make_identity


def _legalize_bir(bir_json: bytes) -> bytes:
    """This container's walrus encodes at most ONE sync wait per instruction
    ('Too many sync wait commands' otherwise), while the Tile scheduler freely
    emits 2+ (and the tail drain waits on every live semaphore). Legalize at
    the BIR level: for every instruction with N>1 waits, insert N-1 NoOps
    carrying one wait each immediately before it on the same engine — the
    engine executes them serially, so blocking semantics are identical."""
    import json

    m = json.loads(bir_json)
    n = 0
    for fn in m["functions"]:
        for bb in fn["blocks"]:
            out = []
            for inst in bb["instructions"]:
                si = inst.get("sync_info")
                waits = (si or {}).get("on_wait", [])
                if si is not None and len(waits) > 1:
                    for w in waits[:-1]:
                        n += 1
                        out.append({
                            "debug": inst.get("debug", 0),
                            "engine": inst["engine"],
                            "ins": [],
                            "outs": [],
                            "name": f"I-sqw-{n}",
                            "opcode": "NoOp",
                            "sync_info": {"on_update": [], "on_wait": [w]},
                        })
                    si["on_wait"] = [waits[-1]]
                out.append(inst)
            bb["instructions"] = out
    return json.dumps(m).encode()


def _install_bir_legalizer():
    import concourse.bass_utils as bu
    import concourse.bass2jax as b2j

    if getattr(bu, "_bir_legalizer_installed", False):
        return
    orig = bu.compile_bir_kernel

    def wrapped(bir_json, tmpdir, neff_name="file.neff"):
        return orig(_legalize_bir(bir_json), tmpdir, neff_name)

    bu.compile_bir_kernel = wrapped
    b2j.compile_bir_kernel = wrapped
    bu._bir_legalizer_installed = True


FP32 = mybir.dt.float32
BF16 = mybir.dt.bfloat16
AF = mybir.ActivationFunctionType
ALU = mybir.AluOpType

B = 8
S = 1024          # 32 * 32
C = 512
NH = 8
HD = 64
INNER = NH * HD   # 512
G = 32            # groups
GS = C // G       # 16 channels per group
EPS = 1e-5
N_CORES = 8

ST = S // 128     # 8 s-tiles
CT = C // 128     # 4 c-tiles
PAIRS = NH // 2   # 4 head pairs
SQC = S // 512    # 2 sq chunks of 512


def _pack_weights(wqkv_f32: np.ndarray, wout_f32: np.ndarray):
    """Host-side packing: w_qkv [C, 3*INNER] -> (w_qk [C, 1024], w_v [C, 512])
    so every matmul stationary slice is contiguous (1 free dim).

    w_qk layout: [c, pair, {q,k}, head_in_pair, d] flattened;
    w_v layout:  [c, head, d] flattened."""
    bf = ml_dtypes.bfloat16
    r = wqkv_f32.reshape(C, NH, 3, HD)
    wqk = (
        r[:, :, 0:2, :]
        .reshape(C, PAIRS, 2, 2, HD)  # [c, pair, h2, j, d]
        .transpose(0, 1, 3, 2, 4)     # [c, pair, j, h2, d]
        .reshape(C, PAIRS * 2 * 2 * HD)
        .astype(bf)
    )
    wv = r[:, :, 2, :].reshape(C, INNER).astype(bf)
    return np.ascontiguousarray(wqk), np.ascontiguousarray(wv), wout_f32.astype(bf)


def _dma(nc, out, in_):
    nc.sync.dma_start(out, in_)


def _emit(nc: bass.Bass, has_bqk: bool, has_bv: bool, has_bout: bool):
    x_d = nc.dram_tensor("x", [S, C], FP32, kind="ExternalInput")
    wqk_d = nc.dram_tensor("w_qk", [C, PAIRS * 2 * 128], BF16, kind="ExternalInput")
    wv_d = nc.dram_tensor("w_v", [C, INNER], BF16, kind="ExternalInput")
    wout_d = nc.dram_tensor("w_out", [INNER, C], BF16, kind="ExternalInput")
    bqkv_d = nc.dram_tensor("b_qkv", [3 * INNER], FP32, kind="ExternalInput")
    bout_d = nc.dram_tensor("b_out", [C], FP32, kind="ExternalInput")
    gns_d = nc.dram_tensor("gn_scale", [C], FP32, kind="ExternalInput")
    gnb_d = nc.dram_tensor("gn_bias", [C], FP32, kind="ExternalInput")
    y_d = nc.dram_tensor("y", [S, C], FP32, kind="ExternalOutput")
    gn_scratch = nc.dram_tensor("gn_scratch", [2, C], FP32, kind="Internal")

    with tile.TileContext(nc) as tc:
        with (
            tc.tile_pool(name="const", bufs=1) as const,
            tc.tile_pool(name="big", bufs=1) as big,
            tc.tile_pool(name="work", bufs=2) as work,
            tc.tile_pool(name="expp", bufs=3) as expp,
            tc.tile_pool(name="ps", bufs=1, space="PSUM") as ps,
        ):
            # ---------- constants / weights ----------
            ident = const.tile([128, 128], BF16)
            make_identity(nc, ident)
            ones_col = const.tile([128, 1], BF16)
            nc.vector.memset(ones_col, 1.0)
            eps_t = const.tile([1, 1], FP32)
            nc.vector.memset(eps_t, EPS)
            actwarm = const.tile([1, 2], FP32)
            nc.scalar.activation(actwarm[:, 0:1], eps_t, AF.Ln)
            nc.scalar.activation(actwarm[:, 1:2], eps_t, AF.Exp)

            # ---------- load x (per-tile DMAs for pipelining) ----------
            x_sb = big.tile([128, ST, C], FP32)
            x_t = x_d.rearrange("(t p) c -> t p c", p=128)
            for t in range(ST):
                _dma(nc, x_sb[:, t], x_t[t])

            wqk_sb = const.tile([128, CT, PAIRS, 2, 128], BF16)
            _dma(nc, 
                wqk_sb,
                wqk_d.rearrange("(ct p) (pr j f) -> p ct pr j f", p=128, pr=PAIRS, j=2),
            )
            wv_sb = const.tile([128, CT, INNER], BF16)
            _dma(nc, wv_sb, wv_d.rearrange("(ct p) f -> p ct f", p=128))
            wout_sb = const.tile([128, CT, C], BF16)
            _dma(nc, wout_sb, wout_d.rearrange("(ct p) f -> p ct f", p=128))
            gamma = const.tile([1, G, GS], FP32)
            _dma(nc, gamma, gns_d[None, :])
            beta = const.tile([1, G, GS], FP32)
            _dma(nc, beta, gnb_d[None, :])

            bqkv_hx = bqkv_d[None, :].rearrange("o (h x) -> o h x", x=3 * HD)
            bqk = bv_bc = bout_bc = None
            if has_bqk:
                # per-partition bias columns for the q/k projections per pair
                bqk = const.tile([128, PAIRS, 2], FP32)
                for p in range(PAIRS):
                    for j in range(2):  # 0 -> q cols, 1 -> k cols
                        _dma(nc,
                            bqk[:, p, j : j + 1],
                            bqkv_hx[0, 2 * p : 2 * p + 2, j * HD : (j + 1) * HD],
                        )
            if has_bv:
                # free-dim bias vector broadcast to 128 partitions via DMA
                # (DRAM-side partition broadcast is allowed, SBUF-side is not)
                bv_bc = const.tile([128, NH, HD], FP32)
                _dma(nc,
                    bv_bc, bqkv_hx[:, :, 2 * HD : 3 * HD].partition_broadcast(128)
                )
            if has_bout:
                bout_bc = const.tile([128, C], FP32)
                _dma(nc, bout_bc, bout_d[None, :].partition_broadcast(128))

            # ---------- GroupNorm stats (the critical chain: emit first so the
            # stats -> scale/bias pipeline outranks the transpose evacuations)
            xbf = big.tile([128, ST, C], BF16)
            xT = [
                big.tile([128, S], BF16, tag=f"xT{ct}", name=f"xT{ct}")
                for ct in range(CT)
            ]
            pstat = ps.tile([1, 2, C], FP32, tag="big2", bufs=3)
            for t in range(ST):
                nc.gpsimd.tensor_copy(xbf[:, t], x_sb[:, t])
                xsq_t = work.tile([128, C], BF16, tag="xsq")
                nc.scalar.activation(xsq_t, xbf[:, t], AF.Square)
                nc.tensor.matmul(
                    pstat[:, 0], ones_col, xbf[:, t], start=(t == 0), stop=(t == ST - 1)
                )
                nc.tensor.matmul(
                    pstat[:, 1], ones_col, xsq_t, start=(t == 0), stop=(t == ST - 1)
                )

            gstat = work.tile([1, 2, G], FP32, tag="gstat")
            nc.vector.reduce_sum(
                gstat,
                pstat.rearrange("p s (g d) -> p s g d", d=GS),
                axis=mybir.AxisListType.X,
            )
            inv_n = 1.0 / (S * GS)
            mean_g = work.tile([1, G], FP32, tag="mean_g")
            nc.vector.tensor_scalar_mul(mean_g, gstat[:, 0], inv_n)
            var_g = work.tile([1, G], FP32, tag="var_g")
            nc.vector.tensor_scalar_mul(var_g, gstat[:, 1], inv_n)
            msq = work.tile([1, G], FP32, tag="msq")
            nc.vector.tensor_mul(msq, mean_g, mean_g)
            nc.vector.tensor_sub(var_g, var_g, msq)
            # rstd = exp(-0.5 * ln(var + eps))  (stays in the exp table set)
            lnv = work.tile([1, G], FP32, tag="lnv")
            nc.scalar.activation(lnv, var_g, AF.Ln, bias=eps_t)
            rstd_g = work.tile([1, G], FP32, tag="rstd_g")
            nc.scalar.activation(rstd_g, lnv, AF.Exp, scale=-0.5)

            # sb_c[0] = scale_c = rstd_g * gamma ; sb_c[1] = bias_c
            sb_c = work.tile([1, 2, G, GS], FP32, tag="sb_c")
            scale_c = sb_c[:, 0]
            bias_c = sb_c[:, 1]
            nc.vector.tensor_tensor(
                scale_c, gamma, rstd_g[:, :, None].to_broadcast([1, G, GS]), ALU.mult
            )
            nc.vector.tensor_tensor(
                bias_c, scale_c, mean_g[:, :, None].to_broadcast([1, G, GS]), ALU.mult
            )
            nc.vector.tensor_sub(bias_c, beta, bias_c)

            # per-partition (channel-major) copies of scale/bias for the
            # transposed-side GN application on ScalarE (DRAM bounce: DMA
            # cannot scatter SBUF free dim -> partitions directly)
            _dma(nc, gn_scratch[None, :], sb_c.rearrange("o s g d -> o s (g d)"))
            sbp = const.tile([128, 2, CT], FP32)
            _dma(nc, sbp, gn_scratch.rearrange("s (ct p) -> p s ct", p=128))
            scale_cp = sbp[:, 0]
            bias_cp = sbp[:, 1]
            # broadcast copies for the natural-layout residual xn
            scale_bc = const.tile([128, C], FP32)
            _dma(nc,
                scale_bc,
                scale_c[:, None, :, :].to_broadcast([1, 128, G, GS]),
            )
            bias_bc = const.tile([128, C], FP32)
            _dma(nc,
                bias_bc,
                bias_c[:, None, :, :].to_broadcast([1, 128, G, GS]),
            )

            # transposes of raw x (independent of the stats chain, lower prio)
            for t in range(ST):
                for ct in range(CT):
                    ptr = ps.tile([128, 128], BF16, tag="acc", bufs=2)
                    nc.tensor.transpose(
                        ptr, xbf[:, t, 128 * ct : 128 * (ct + 1)], ident
                    )
                    nc.vector.tensor_copy(xT[ct][:, 128 * t : 128 * (t + 1)], ptr)

            # ---------- GN on the transposed side: xnT = xT*scale + bias ----
            xnT = [
                big.tile([128, S], BF16, tag=f"xnT{ct}", name=f"xnT{ct}")
                for ct in range(CT)
            ]
            for ct in range(CT):
                nc.scalar.activation(
                    xnT[ct],
                    xT[ct],
                    AF.Identity,
                    bias=bias_cp[:, ct : ct + 1],
                    scale=scale_cp[:, ct : ct + 1],
                )

            # ---------- per head pair: q/k projections, then attention ----------
            # outT is split per (pair, sq-chunk) so the output projection can
            # start as soon as the chunk it reads is normalized.
            outT = [
                [
                    big.tile(
                        [128, 512], BF16, tag=f"outT{p}_{cc}", name=f"outT{p}_{cc}"
                    )
                    for cc in range(SQC)
                ]
                for p in range(PAIRS)
            ]

            def alloc_qk(p):
                qT = work.tile([128, S], BF16, tag="qT", name=f"qT{p}")
                kT = work.tile([128, S], BF16, tag="kT", name=f"kT{p}")
                return qT, kT

            def emit_qk_group(p, qT, kT, j, cchunk):
                dst = (qT, kT)[j]
                pqk = ps.tile([128, 512], FP32, tag="acc", bufs=2)
                for ct in range(CT):
                    nc.tensor.matmul(
                        pqk,
                        wqk_sb[:, ct, p, j],
                        xnT[ct][:, 512 * cchunk : 512 * (cchunk + 1)],
                        start=(ct == 0),
                        stop=(ct == CT - 1),
                    )
                if has_bqk:
                    nc.vector.tensor_scalar(
                        dst[:, 512 * cchunk : 512 * (cchunk + 1)],
                        pqk,
                        bqk[:, p, j : j + 1],
                        None,
                        ALU.add,
                    )
                else:
                    nc.vector.tensor_copy(
                        dst[:, 512 * cchunk : 512 * (cchunk + 1)], pqk
                    )

            qk_tiles = {0: alloc_qk(0)}
            for j in (0, 1):
                for cc in range(SQC):
                    emit_qk_group(0, *qk_tiles[0], j, cc)

            # ---------- v projection (natural layout) + ones column ----------
            # emitted lazily inside pair-0/head-0's loop so the first QK
            # matmuls outrank it in the PE's static order
            v_sb = big.tile([128, ST, NH, HD + 1], BF16)

            def emit_v(t):
                pv = ps.tile([128, INNER], FP32, tag="acc", bufs=2)
                for ct in range(CT):
                    nc.tensor.matmul(
                        pv,
                        xnT[ct][:, 128 * t : 128 * (t + 1)],
                        wv_sb[:, ct],
                        start=(ct == 0),
                        stop=(ct == CT - 1),
                    )
                if has_bv:
                    nc.vector.tensor_tensor(
                        v_sb[:, t, :, 0:HD],
                        pv.rearrange("p (h d) -> p h d", d=HD),
                        bv_bc,
                        ALU.add,
                    )
                else:
                    nc.vector.tensor_copy(
                        v_sb[:, t, :, 0:HD], pv.rearrange("p (h d) -> p h d", d=HD)
                    )
                nc.gpsimd.memset(v_sb[:, t, :, HD : HD + 1], 1.0)

            # ---------- natural-layout xn for the residual (off critical path)
            xn = big.tile([128, ST, C], FP32)
            for t in range(ST):
                nc.vector.tensor_tensor(xn[:, t], x_sb[:, t], scale_bc, ALU.mult)
                nc.vector.tensor_tensor(xn[:, t], xn[:, t], bias_bc, ALU.add)

            for p in range(PAIRS):
                qT, kT = qk_tiles.pop(p)
                if p + 1 < PAIRS:
                    qk_tiles[p + 1] = alloc_qk(p + 1)
                    next_groups = [(j, cc) for j in (0, 1) for cc in range(SQC)]
                # attention, one head at a time. The PE is in-order, so AV
                # matmuls (which wait on exp) are emitted with a 2-step lag
                # behind the QK matmuls: the exp chain on ScalarE then never
                # waits for the PE.
                for h in range(2):
                    po_h = ps.tile(
                        [HD + 1, SQC, 512], FP32, tag="big2", bufs=3, name=f"po{p}_{h}"
                    )

                    def emit_av(skt, et):
                        for cc in range(SQC):
                            nc.tensor.matmul(
                                po_h[:, cc, :],
                                v_sb[:, skt, 2 * p + h],
                                et[:, 512 * cc : 512 * (cc + 1)],
                                start=(skt == 0),
                                stop=(skt == ST - 1),
                            )

                    pend = []
                    for skt in range(ST):
                        psc = ps.tile([128, S], FP32, tag="big2", bufs=3)
                        for cc in range(SQC):
                            nc.tensor.matmul(
                                psc[:, 512 * cc : 512 * (cc + 1)],
                                kT[64 * h : 64 * (h + 1), 128 * skt : 128 * (skt + 1)],
                                qT[64 * h : 64 * (h + 1), 512 * cc : 512 * (cc + 1)],
                                start=True,
                                stop=True,
                            )
                        et = expp.tile([128, S], BF16, tag="exp", bufs=12)
                        nc.scalar.activation(et, psc, AF.Exp, scale=1.0 / 8.0)
                        if p == 0 and h == 0:
                            emit_v(skt)
                        if h == 1 and p + 1 < PAIRS and skt % 2 == 0:
                            j, cc = next_groups[skt // 2]
                            emit_qk_group(p + 1, *qk_tiles[p + 1], j, cc)
                        pend.append((skt, et))
                        if len(pend) > 4:
                            emit_av(*pend.pop(0))
                    for item in pend:
                        emit_av(*item)
                    # normalize: out / rowsum (rowsum is PSUM row 64).
                    # 1/r = exp(-ln(r)) on ScalarE: a 1-lane DVE reciprocal is
                    # ~8 cycles/element (4us per row on HW) while ACT runs
                    # 1 cycle/element and reads PSUM directly. The fp32
                    # reciprocal row is then broadcast across the 64 head-dim
                    # partitions with a stride-0 SBUF->SBUF DMA.
                    lnr = work.tile([1, SQC, 512], FP32, tag="lnr")
                    nc.scalar.activation(lnr, po_h[HD : HD + 1, :, :], AF.Ln)
                    recip = work.tile([1, SQC, 512], FP32, tag="recip")
                    nc.scalar.activation(recip, lnr, AF.Exp, scale=-1.0)
                    pou = work.tile([HD, SQC, 512], FP32, tag="pou")
                    nc.vector.tensor_copy(pou, po_h[0:HD, :, :])
                    for cc in range(SQC):
                        rbc = work.tile([HD, 512], FP32, tag="rbc")
                        _dma(nc,
                            rbc,
                            recip[:, cc][:, None, :].to_broadcast([1, HD, 512]),
                        )
                        nc.vector.tensor_tensor(
                            outT[p][cc][64 * h : 64 * (h + 1), :],
                            pou[:, cc, :],
                            rbc,
                            ALU.mult,
                        )

            # ---------- output projection + residual ----------
            for t in range(ST):
                ph = ps.tile([128, C], FP32, tag="acc", bufs=2)
                for p in range(PAIRS):
                    nc.tensor.matmul(
                        ph,
                        outT[p][t // 4][:, 128 * (t % 4) : 128 * (t % 4 + 1)],
                        wout_sb[:, p],
                        start=(p == 0),
                        stop=(p == PAIRS - 1),
                    )
                yt = work.tile([128, C], FP32, tag="y")
                nc.vector.tensor_tensor(yt, ph, xn[:, t], ALU.add)
                if has_bout:
                    nc.vector.tensor_tensor(yt, yt, bout_bc, ALU.add)
                _dma(nc, y_d[128 * t : 128 * (t + 1), :], yt)

    return nc


_CACHE: dict = {}


def _get_program(has_bqk: bool = True, has_bv: bool = True, has_bout: bool = True) -> bass.Bass:
    key = (has_bqk, has_bv, has_bout)
    if key not in _CACHE:
        _install_bir_legalizer()
        nc = bass.Bass()
        _emit(nc, has_bqk, has_bv, has_bout)
        _CACHE[key] = nc
    return _CACHE[key]


def _get_runner(nc: bass.Bass):
    """Build the sharded PJRT executable once and cache it — bass2jax's
    run_bass_via_pjrt re-jits a fresh closure every call (~1.5 s of host-side
    retrace per invocation)."""
    if "runner" in _CACHE and _CACHE["runner"][0] is nc:
        return _CACHE["runner"][1]

    import jax
    from jax.sharding import Mesh, PartitionSpec
    from jax.experimental.shard_map import shard_map
    from concourse import bass2jax, mybir as _mb

    bass2jax.install_neuronx_cc_hook()
    partition_name = (
        nc.partition_id_tensor.name if nc.partition_id_tensor else None
    )
    in_names, out_names, out_avals, zero_outs = [], [], [], []
    for alloc in nc.m.functions[0].allocations:
        if not isinstance(alloc, _mb.MemoryLocationSet):
            continue
        name = alloc.memorylocations[0].name
        if alloc.kind == "ExternalInput":
            if name != partition_name:
                in_names.append(name)
        elif alloc.kind == "ExternalOutput":
            shape = tuple(alloc.tensor_shape)
            dtype = _mb.dt.np(alloc.dtype)
            out_names.append(name)
            out_avals.append(jax.core.ShapedArray(shape, dtype))
            zero_outs.append(np.zeros(shape, dtype))
    n_params = len(in_names)
    n_outs = len(out_avals)
    all_in_names = list(in_names) + list(out_names)
    if partition_name is not None:
        all_in_names.append(partition_name)
    donate = tuple(range(n_params, n_params + n_outs))

    def _body(*args):
        operands = list(args)
        if partition_name is not None:
            operands.append(bass2jax.partition_id_tensor())
        outs = bass2jax._bass_exec_p.bind(
            *operands,
            out_avals=tuple(out_avals),
            in_names=tuple(all_in_names),
            out_names=tuple(out_names),
            lowering_input_output_aliases=(),
            sim_require_finite=True,
            sim_require_nnan=True,
            nc=nc,
        )
        return tuple(outs)

    devices = jax.devices()[:N_CORES]
    mesh = Mesh(np.asarray(devices), ("core",))
    in_specs = (PartitionSpec("core"),) * (n_params + n_outs)
    out_specs = (PartitionSpec("core"),) * n_outs
    sharded = jax.jit(
        shard_map(
            _body, mesh=mesh, in_specs=in_specs, out_specs=out_specs,
            check_rep=False,
        ),
        donate_argnums=donate,
        keep_unused=True,
    )

    def run(in_maps):
        per_core = [[np.asarray(m[name]) for name in in_names] for m in in_maps]
        concat_in = [
            np.concatenate([per_core[c][i] for c in range(N_CORES)], axis=0)
            for i in range(n_params)
        ]
        concat_zeros = [
            np.zeros((N_CORES * z.shape[0], *z.shape[1:]), z.dtype)
            for z in zero_outs
        ]
        out_arrs = sharded(*concat_in, *concat_zeros)
        return [
            {
                name: np.asarray(out_arrs[i]).reshape(
                    N_CORES, *out_avals[i].shape
                )[c]
                for i, name in enumerate(out_names)
            }
            for c in range(N_CORES)
        ]

    _CACHE["runner"] = (nc, run)
    return run


def kernel(**inputs) -> np.ndarray:
    x = np.asarray(inputs["x"], dtype=np.float32)
    wqk, wv, wout = _pack_weights(
        np.asarray(inputs["w_qkv"], dtype=np.float32),
        np.asarray(inputs["w_out"], dtype=np.float32),
    )
    common = {
        "w_qk": wqk,
        "w_v": wv,
        "w_out": wout,
        "b_qkv": np.asarray(inputs["b_qkv"], dtype=np.float32),
        "b_out": np.asarray(inputs["b_out"], dtype=np.float32),
        "gn_scale": np.asarray(inputs["gn_scale"], dtype=np.float32),
        "gn_bias": np.asarray(inputs["gn_bias"], dtype=np.float32),
    }
    in_maps = [
        {"x": np.ascontiguousarray(x[i].reshape(S, C)), **common} for i in range(B)
    ]
    bq = common["b_qkv"]
    nc = _get_program(
        has_bqk=bool(np.any(bq.reshape(NH, 3, HD)[:, 0:2, :])),
        has_bv=bool(np.any(bq.reshape(NH, 3, HD)[:, 2, :])),
        has_bout=bool(np.any(common["b_out"])),
    )
    res = _get_runner(nc)(in_maps)
    y = np.stack([r["y"] for r in res], axis=0)
    return y.reshape(B, 32, 32, C).astype(np.float32)


# revision 51
# speedup vs baseline: 1.0464x; 1.0125x over previous
"""Trainium2 Bass kernel for an AttentionBlock (GroupNorm + MHA + residual).

Shapes (hardcoded from the problem spec):
  x: [8, 32, 32, 512] fp32, t: [8] (unused by the reference),
  gn_scale/gn_bias: [512], w_qkv: [512, 1536], b_qkv: [1536],
  w_out: [512, 512], b_out: [512].

Sharding: pure data parallel — one batch element per NeuronCore (8 cores).
Each core runs the full block on its [1024, 512] slice; no collectives.

Per-core compute layout:
  - GroupNorm stats via ones-vector matmuls (sum over s on the PE),
    rsqrt as exp(-0.5*ln(var+eps)) so ScalarE stays in one table set.
  - xn transposed to [c, s] with PE transposes; qT/kT computed per head-pair
    [128, 1024] so QK^T row-packs two K=64 heads onto the PE array.
  - scores are computed transposed ([sk, sq]) so the softmax denominator and
    the attention*V contraction both come out of matmuls without transposing
    the [1024, 1024] attention matrix: AV uses v_aug=[v | 1] (M=65) stationary,
    and row 64 of the PSUM accumulator is the softmax row-sum.
  - exp runs on ScalarE straight out of PSUM, bf16 out.
  - matmul operands are bf16 (fp32 matmuls are 4x slower on TRN2); all
    accumulation is fp32 in PSUM; GroupNorm/normalize/residual math is fp32.

PSUM budget (8 banks, statically reserved by tile tags):
  psc [128,1024] f32 x1 = 2 banks (also used for the GN stats accumulator),
  acc [128,512] f32 x2 = 2 banks (transposes, qkv/out-proj accumulators),
  po  [128,512] f32 x4 = 4 banks (per-pair AV accumulators, 2 heads x 2 chunks).
"""

import numpy as np
import ml_dtypes

import concourse.bass as bass
import concourse.mybir as mybir
import concourse.tile as tile
from concourse.masks import make_identity


def _legalize_bir(bir_json: bytes) -> bytes:
    """This container's walrus encodes at most ONE sync wait per instruction
    ('Too many sync wait commands' otherwise), while the Tile scheduler freely
    emits 2+ (and the tail drain waits on every live semaphore). Legalize at
    the BIR level: for every instruction with N>1 waits, insert N-1 NoOps
    carrying one wait each immediately before it on the same engine — the
    engine executes them serially, so blocking semantics are identical."""
    import json

    m = json.loads(bir_json)
    n = 0
    for fn in m["functions"]:
        for bb in fn["blocks"]:
            out = []
            for inst in bb["instructions"]:
                si = inst.get("sync_info")
                waits = (si or {}).get("on_wait", [])
                if si is not None and len(waits) > 1:
                    for w in waits[:-1]:
                        n += 1
                        out.append({
                            "debug": inst.get("debug", 0),
                            "engine": inst["engine"],
                            "ins": [],
                            "outs": [],
                            "name": f"I-sqw-{n}",
                            "opcode": "NoOp",
                            "sync_info": {"on_update": [], "on_wait": [w]},
                        })
                    si["on_wait"] = [waits[-1]]
                out.append(inst)
            bb["instructions"] = out
    return json.dumps(m).encode()


def _install_bir_legalizer():
    import concourse.bass_utils as bu
    import concourse.bass2jax as b2j

    if getattr(bu, "_bir_legalizer_installed", False):
        return
    orig = bu.compile_bir_kernel

    def wrapped(bir_json, tmpdir, neff_name="file.neff"):
        return orig(_legalize_bir(bir_json), tmpdir, neff_name)

    bu.compile_bir_kernel = wrapped
    b2j.compile_bir_kernel = wrapped
    bu._bir_legalizer_installed = True


FP32 = mybir.dt.float32
BF16 = mybir.dt.bfloat16
AF = mybir.ActivationFunctionType
ALU = mybir.AluOpType

B = 8
S = 1024          # 32 * 32
C = 512
NH = 8
HD = 64
INNER = NH * HD   # 512
G = 32            # groups
GS = C // G       # 16 channels per group
EPS = 1e-5
N_CORES = 8

ST = S // 128     # 8 s-tiles
CT = C // 128     # 4 c-tiles
PAIRS = NH // 2   # 4 head pairs
SQC = S // 512    # 2 sq chunks of 512


def _pack_weights(wqkv_f32: np.ndarray, wout_f32: np.ndarray):
    """Host-side packing: w_qkv [C, 3*INNER] -> (w_qk [C, 1024], w_v [C, 512])
    so every matmul stationary slice is contiguous (1 free dim).

    w_qk layout: [c, pair, {q,k}, head_in_pair, d] flattened;
    w_v layout:  [c, head, d] flattened."""
    bf = ml_dtypes.bfloat16
    r = wqkv_f32.reshape(C, NH, 3, HD)
    wqk = (
        r[:, :, 0:2, :]
        .reshape(C, PAIRS, 2, 2, HD)  # [c, pair, h2, j, d]
        .transpose(0, 1, 3, 2, 4)     # [c, pair, j, h2, d]
        .reshape(C, PAIRS * 2 * 2 * HD)
        .astype(bf)
    )
    wv = r[:, :, 2, :].reshape(C, INNER).astype(bf)
    return np.ascontiguousarray(wqk), np.ascontiguousarray(wv), wout_f32.astype(bf)


def _dma(nc, out, in_):
    nc.sync.dma_start(out, in_)


def _emit(nc: bass.Bass, has_bqk: bool, has_bv: bool, has_bout: bool):
    x_d = nc.dram_tensor("x", [S, C], FP32, kind="ExternalInput")
    wqk_d = nc.dram_tensor("w_qk", [C, PAIRS * 2 * 128], BF16, kind="ExternalInput")
    wv_d = nc.dram_tensor("w_v", [C, INNER], BF16, kind="ExternalInput")
    wout_d = nc.dram_tensor("w_out", [INNER, C], BF16, kind="ExternalInput")
    bqkv_d = nc.dram_tensor("b_qkv", [3 * INNER], FP32, kind="ExternalInput")
    bout_d = nc.dram_tensor("b_out", [C], FP32, kind="ExternalInput")
    gns_d = nc.dram_tensor("gn_scale", [C], FP32, kind="ExternalInput")
    gnb_d = nc.dram_tensor("gn_bias", [C], FP32, kind="ExternalInput")
    y_d = nc.dram_tensor("y", [S, C], FP32, kind="ExternalOutput")
    gn_scratch = nc.dram_tensor("gn_scratch", [2, C], FP32, kind="Internal")

    with tile.TileContext(nc) as tc:
        with (
            tc.tile_pool(name="const", bufs=1) as const,
            tc.tile_pool(name="big", bufs=1) as big,
            tc.tile_pool(name="work", bufs=2) as work,
            tc.tile_pool(name="expp", bufs=3) as expp,
            tc.tile_pool(name="ps", bufs=1, space="PSUM") as ps,
        ):
            # ---------- constants / weights ----------
            ident = const.tile([128, 128], BF16)
            make_identity(nc, ident)
            ones_col = const.tile([128, 1], BF16)
            nc.vector.memset(ones_col, 1.0)
            eps_t = const.tile([1, 1], FP32)
            nc.vector.memset(eps_t, EPS)
            ones_r64 = const.tile([1, HD], FP32)
            nc.vector.memset(ones_r64, 1.0)
            ones_r128 = const.tile([1, 128], FP32)
            nc.vector.memset(ones_r128, 1.0)
            actwarm = const.tile([1, 2], FP32)
            nc.scalar.activation(actwarm[:, 0:1], eps_t, AF.Ln)
            nc.scalar.activation(actwarm[:, 1:2], eps_t, AF.Exp)

            # ---------- load x (per-tile DMAs for pipelining) ----------
            x_sb = big.tile([128, ST, C], FP32)
            x_t = x_d.rearrange("(t p) c -> t p c", p=128)
            for t in range(ST):
                _dma(nc, x_sb[:, t], x_t[t])

            wqk_sb = const.tile([128, CT, PAIRS, 2, 128], BF16)
            _dma(nc, 
                wqk_sb,
                wqk_d.rearrange("(ct p) (pr j f) -> p ct pr j f", p=128, pr=PAIRS, j=2),
            )
            wv_sb = const.tile([128, CT, INNER], BF16)
            _dma(nc, wv_sb, wv_d.rearrange("(ct p) f -> p ct f", p=128))
            wout_sb = const.tile([128, CT, C], BF16)
            _dma(nc, wout_sb, wout_d.rearrange("(ct p) f -> p ct f", p=128))
            gamma = const.tile([1, G, GS], FP32)
            _dma(nc, gamma, gns_d[None, :])
            beta = const.tile([1, G, GS], FP32)
            _dma(nc, beta, gnb_d[None, :])

            bqkv_hx = bqkv_d[None, :].rearrange("o (h x) -> o h x", x=3 * HD)
            bqk = bv_bc = bout_bc = None
            if has_bqk:
                # per-partition bias columns for the q/k projections per pair
                bqk = const.tile([128, PAIRS, 2], FP32)
                for p in range(PAIRS):
                    for j in range(2):  # 0 -> q cols, 1 -> k cols
                        _dma(nc,
                            bqk[:, p, j : j + 1],
                            bqkv_hx[0, 2 * p : 2 * p + 2, j * HD : (j + 1) * HD],
                        )
            if has_bv:
                # free-dim bias vector broadcast to 128 partitions via DMA
                # (DRAM-side partition broadcast is allowed, SBUF-side is not)
                bv_bc = const.tile([128, NH, HD], FP32)
                _dma(nc,
                    bv_bc, bqkv_hx[:, :, 2 * HD : 3 * HD].partition_broadcast(128)
                )
            if has_bout:
                bout_bc = const.tile([128, C], FP32)
                _dma(nc, bout_bc, bout_d[None, :].partition_broadcast(128))

            # ---------- GroupNorm stats (the critical chain: emit first so the
            # stats -> scale/bias pipeline outranks the transpose evacuations)
            xbf = big.tile([128, ST, C], BF16)
            xT = [
                big.tile([128, S], BF16, tag=f"xT{ct}", name=f"xT{ct}")
                for ct in range(CT)
            ]
            pstat = ps.tile([1, 2, C], FP32, tag="big2", bufs=3)
            for t in range(ST):
                nc.gpsimd.tensor_copy(xbf[:, t], x_sb[:, t])
                xsq_t = work.tile([128, C], BF16, tag="xsq")
                nc.scalar.activation(xsq_t, xbf[:, t], AF.Square)
                nc.tensor.matmul(
                    pstat[:, 0], ones_col, xbf[:, t], start=(t == 0), stop=(t == ST - 1)
                )
                nc.tensor.matmul(
                    pstat[:, 1], ones_col, xsq_t, start=(t == 0), stop=(t == ST - 1)
                )

            gstat = work.tile([1, 2, G], FP32, tag="gstat")
            nc.vector.reduce_sum(
                gstat,
                pstat.rearrange("p s (g d) -> p s g d", d=GS),
                axis=mybir.AxisListType.X,
            )
            inv_n = 1.0 / (S * GS)
            mean_g = work.tile([1, G], FP32, tag="mean_g")
            nc.vector.tensor_scalar_mul(mean_g, gstat[:, 0], inv_n)
            var_g = work.tile([1, G], FP32, tag="var_g")
            nc.vector.tensor_scalar_mul(var_g, gstat[:, 1], inv_n)
            msq = work.tile([1, G], FP32, tag="msq")
            nc.vector.tensor_mul(msq, mean_g, mean_g)
            nc.vector.tensor_sub(var_g, var_g, msq)
            # rstd = exp(-0.5 * ln(var + eps))  (stays in the exp table set)
            lnv = work.tile([1, G], FP32, tag="lnv")
            nc.scalar.activation(lnv, var_g, AF.Ln, bias=eps_t)
            rstd_g = work.tile([1, G], FP32, tag="rstd_g")
            nc.scalar.activation(rstd_g, lnv, AF.Exp, scale=-0.5)

            # sb_c[0] = scale_c = rstd_g * gamma ; sb_c[1] = bias_c
            sb_c = work.tile([1, 2, G, GS], FP32, tag="sb_c")
            scale_c = sb_c[:, 0]
            bias_c = sb_c[:, 1]
            nc.vector.tensor_tensor(
                scale_c, gamma, rstd_g[:, :, None].to_broadcast([1, G, GS]), ALU.mult
            )
            nc.vector.tensor_tensor(
                bias_c, scale_c, mean_g[:, :, None].to_broadcast([1, G, GS]), ALU.mult
            )
            nc.vector.tensor_sub(bias_c, beta, bias_c)

            # per-partition (channel-major) copies of scale/bias for the
            # transposed-side GN application on ScalarE (DRAM bounce: DMA
            # cannot scatter SBUF free dim -> partitions directly)
            _dma(nc, gn_scratch[None, :], sb_c.rearrange("o s g d -> o s (g d)"))
            sbp = const.tile([128, 2, CT], FP32)
            _dma(nc, sbp, gn_scratch.rearrange("s (ct p) -> p s ct", p=128))
            scale_cp = sbp[:, 0]
            bias_cp = sbp[:, 1]
            # broadcast copies for the natural-layout residual xn: K=1
            # ones-matmuls (a stride-0 partition-broadcast DMA runs at ~2GB/s)
            sb_bc = ps.tile([128, 2, C], FP32, tag="big2", bufs=3, name="sb_bc")
            nc.tensor.matmul(
                sb_bc[:, 0], ones_r128, sb_c.rearrange("o s g d -> o (s g d)")[:, 0:C],
                start=True, stop=True,
            )
            nc.tensor.matmul(
                sb_bc[:, 1], ones_r128,
                sb_c.rearrange("o s g d -> o (s g d)")[:, C : 2 * C],
                start=True, stop=True,
            )
            scale_bc = sb_bc[:, 0]
            bias_bc = sb_bc[:, 1]

            # transposes of raw x (independent of the stats chain, lower prio)
            for t in range(ST):
                for ct in range(CT):
                    ptr = ps.tile([128, 128], BF16, tag="acc", bufs=2)
                    nc.tensor.transpose(
                        ptr, xbf[:, t, 128 * ct : 128 * (ct + 1)], ident
                    )
                    nc.vector.tensor_copy(xT[ct][:, 128 * t : 128 * (t + 1)], ptr)

            # ---------- GN on the transposed side: xnT = xT*scale + bias ----
            xnT = [
                big.tile([128, S], BF16, tag=f"xnT{ct}", name=f"xnT{ct}")
                for ct in range(CT)
            ]
            for ct in range(CT):
                nc.scalar.activation(
                    xnT[ct],
                    xT[ct],
                    AF.Identity,
                    bias=bias_cp[:, ct : ct + 1],
                    scale=scale_cp[:, ct : ct + 1],
                )

            # ---------- per head pair: q/k projections, then attention ----------
            # outT is split per (pair, sq-chunk) so the output projection can
            # start as soon as the chunk it reads is normalized.
            outT = [
                [
                    big.tile(
                        [128, 512], BF16, tag=f"outT{p}_{cc}", name=f"outT{p}_{cc}"
                    )
                    for cc in range(SQC)
                ]
                for p in range(PAIRS)
            ]

            def alloc_qk(p):
                qT = work.tile([128, S], BF16, tag="qT", name=f"qT{p}")
                kT = work.tile([128, S], BF16, tag="kT", name=f"kT{p}")
                return qT, kT

            def emit_qk_group(p, qT, kT, j, cchunk):
                dst = (qT, kT)[j]
                pqk = ps.tile([128, 512], FP32, tag="acc", bufs=2)
                for ct in range(CT):
                    nc.tensor.matmul(
                        pqk,
                        wqk_sb[:, ct, p, j],
                        xnT[ct][:, 512 * cchunk : 512 * (cchunk + 1)],
                        start=(ct == 0),
                        stop=(ct == CT - 1),
                    )
                if has_bqk:
                    nc.vector.tensor_scalar(
                        dst[:, 512 * cchunk : 512 * (cchunk + 1)],
                        pqk,
                        bqk[:, p, j : j + 1],
                        None,
                        ALU.add,
                    )
                else:
                    nc.vector.tensor_copy(
                        dst[:, 512 * cchunk : 512 * (cchunk + 1)], pqk
                    )

            qk_tiles = {0: alloc_qk(0)}
            for j in (0, 1):
                for cc in range(SQC):
                    emit_qk_group(0, *qk_tiles[0], j, cc)

            # ---------- v projection (natural layout) + ones column ----------
            # emitted lazily inside pair-0/head-0's loop so the first QK
            # matmuls outrank it in the PE's static order
            v_sb = big.tile([128, ST, NH, HD + 1], BF16)

            def emit_v(t):
                pv = ps.tile([128, INNER], FP32, tag="acc", bufs=2)
                for ct in range(CT):
                    nc.tensor.matmul(
                        pv,
                        xnT[ct][:, 128 * t : 128 * (t + 1)],
                        wv_sb[:, ct],
                        start=(ct == 0),
                        stop=(ct == CT - 1),
                    )
                if has_bv:
                    nc.vector.tensor_tensor(
                        v_sb[:, t, :, 0:HD],
                        pv.rearrange("p (h d) -> p h d", d=HD),
                        bv_bc,
                        ALU.add,
                    )
                else:
                    nc.vector.tensor_copy(
                        v_sb[:, t, :, 0:HD], pv.rearrange("p (h d) -> p h d", d=HD)
                    )
                nc.gpsimd.memset(v_sb[:, t, :, HD : HD + 1], 1.0)

            # ---------- natural-layout xn for the residual (off critical path)
            sb_bc_sb = big.tile([128, 2, C], FP32)
            nc.vector.tensor_copy(sb_bc_sb, sb_bc)
            xn = big.tile([128, ST, C], FP32)
            for t in range(ST):
                nc.vector.tensor_tensor(
                    xn[:, t], x_sb[:, t], sb_bc_sb[:, 0], ALU.mult
                )
                nc.vector.tensor_tensor(
                    xn[:, t], xn[:, t], sb_bc_sb[:, 1], ALU.add
                )

            for p in range(PAIRS):
                qT, kT = qk_tiles.pop(p)
                if p + 1 < PAIRS:
                    qk_tiles[p + 1] = alloc_qk(p + 1)
                    next_groups = [(j, cc) for j in (0, 1) for cc in range(SQC)]
                # attention, one head at a time. The PE is in-order, so AV
                # matmuls (which wait on exp) are emitted with a 2-step lag
                # behind the QK matmuls: the exp chain on ScalarE then never
                # waits for the PE.
                for h in range(2):
                    po_h = ps.tile(
                        [HD + 1, SQC, 512], FP32, tag="big2", bufs=3, name=f"po{p}_{h}"
                    )

                    def emit_av(skt, et):
                        for cc in range(SQC):
                            nc.tensor.matmul(
                                po_h[:, cc, :],
                                v_sb[:, skt, 2 * p + h],
                                et[:, 512 * cc : 512 * (cc + 1)],
                                start=(skt == 0),
                                stop=(skt == ST - 1),
                            )

                    pend = []
                    for skt in range(ST):
                        psc = ps.tile([128, S], FP32, tag="big2", bufs=3)
                        for cc in range(SQC):
                            nc.tensor.matmul(
                                psc[:, 512 * cc : 512 * (cc + 1)],
                                kT[64 * h : 64 * (h + 1), 128 * skt : 128 * (skt + 1)],
                                qT[64 * h : 64 * (h + 1), 512 * cc : 512 * (cc + 1)],
                                start=True,
                                stop=True,
                            )
                        et = expp.tile([128, S], BF16, tag="exp", bufs=12)
                        nc.scalar.activation(et, psc, AF.Exp, scale=1.0 / 8.0)
                        if p == 0 and h == 0:
                            emit_v(skt)
                        if h == 1 and p + 1 < PAIRS and skt % 2 == 0:
                            j, cc = next_groups[skt // 2]
                            emit_qk_group(p + 1, *qk_tiles[p + 1], j, cc)
                        pend.append((skt, et))
                        if len(pend) > 4:
                            emit_av(*pend.pop(0))
                    for item in pend:
                        emit_av(*item)
                    # normalize: out / rowsum (rowsum is PSUM row 64).
                    # 1/r = exp(-ln(r)) on ScalarE: a 1-lane DVE reciprocal is
                    # ~8 cycles/element (4us per row on HW) while ACT runs
                    # 1 cycle/element and reads PSUM directly. The fp32
                    # reciprocal row is then broadcast across the 64 head-dim
                    # partitions with a stride-0 SBUF->SBUF DMA.
                    lnr = work.tile([1, SQC, 512], FP32, tag="lnr")
                    nc.scalar.activation(lnr, po_h[HD : HD + 1, :, :], AF.Ln)
                    recip = work.tile([1, SQC, 512], FP32, tag="recip")
                    nc.scalar.activation(recip, lnr, AF.Exp, scale=-1.0)
                    pou = work.tile([HD, SQC, 512], FP32, tag="pou")
                    nc.vector.tensor_copy(pou, po_h[0:HD, :, :])
                    for cc in range(SQC):
                        rbc = ps.tile(
                            [HD, 512], FP32, tag="acc", bufs=2, name=f"rbc{h}_{cc}"
                        )
                        nc.tensor.matmul(
                            rbc, ones_r64, recip[:, cc], start=True, stop=True
                        )
                        nc.vector.tensor_tensor(
                            outT[p][cc][64 * h : 64 * (h + 1), :],
                            pou[:, cc, :],
                            rbc,
                            ALU.mult,
                        )

            # ---------- output projection + residual ----------
            for t in range(ST):
                ph = ps.tile([128, C], FP32, tag="acc", bufs=2)
                for p in range(PAIRS):
                    nc.tensor.matmul(
                        ph,
                        outT[p][t // 4][:, 128 * (t % 4) : 128 * (t % 4 + 1)],
                        wout_sb[:, p],
                        start=(p == 0),
                        stop=(p == PAIRS - 1),
                    )
                yt = work.tile([128, C], FP32, tag="y")
                nc.vector.tensor_tensor(yt, ph, xn[:, t], ALU.add)
                if has_bout:
                    nc.vector.tensor_tensor(yt, yt, bout_bc, ALU.add)
                _dma(nc, y_d[128 * t : 128 * (t + 1), :], yt)

    return nc


_CACHE: dict = {}


def _get_program(has_bqk: bool = True, has_bv: bool = True, has_bout: bool = True) -> bass.Bass:
    key = (has_bqk, has_bv, has_bout)
    if key not in _CACHE:
        _install_bir_legalizer()
        nc = bass.Bass()
        _emit(nc, has_bqk, has_bv, has_bout)
        _CACHE[key] = nc
    return _CACHE[key]


def _get_runner(nc: bass.Bass):
    """Build the sharded PJRT executable once and cache it — bass2jax's
    run_bass_via_pjrt re-jits a fresh closure every call (~1.5 s of host-side
    retrace per invocation)."""
    if "runner" in _CACHE and _CACHE["runner"][0] is nc:
        return _CACHE["runner"][1]

    import jax
    from jax.sharding import Mesh, PartitionSpec
    from jax.experimental.shard_map import shard_map
    from concourse import bass2jax, mybir as _mb

    bass2jax.install_neuronx_cc_hook()
    partition_name = (
        nc.partition_id_tensor.name if nc.partition_id_tensor else None
    )
    in_names, out_names, out_avals, zero_outs = [], [], [], []
    for alloc in nc.m.functions[0].allocations:
        if not isinstance(alloc, _mb.MemoryLocationSet):
            continue
        name = alloc.memorylocations[0].name
        if alloc.kind == "ExternalInput":
            if name != partition_name:
                in_names.append(name)
        elif alloc.kind == "ExternalOutput":
            shape = tuple(alloc.tensor_shape)
            dtype = _mb.dt.np(alloc.dtype)
            out_names.append(name)
            out_avals.append(jax.core.ShapedArray(shape, dtype))
            zero_outs.append(np.zeros(shape, dtype))
    n_params = len(in_names)
    n_outs = len(out_avals)
    all_in_names = list(in_names) + list(out_names)
    if partition_name is not None:
        all_in_names.append(partition_name)
    donate = tuple(range(n_params, n_params + n_outs))

    def _body(*args):
        operands = list(args)
        if partition_name is not None:
            operands.append(bass2jax.partition_id_tensor())
        outs = bass2jax._bass_exec_p.bind(
            *operands,
            out_avals=tuple(out_avals),
            in_names=tuple(all_in_names),
            out_names=tuple(out_names),
            lowering_input_output_aliases=(),
            sim_require_finite=True,
            sim_require_nnan=True,
            nc=nc,
        )
        return tuple(outs)

    devices = jax.devices()[:N_CORES]
    mesh = Mesh(np.asarray(devices), ("core",))
    in_specs = (PartitionSpec("core"),) * (n_params + n_outs)
    out_specs = (PartitionSpec("core"),) * n_outs
    sharded = jax.jit(
        shard_map(
            _body, mesh=mesh, in_specs=in_specs, out_specs=out_specs,
            check_rep=False,
        ),
        donate_argnums=donate,
        keep_unused=True,
    )

    def run(in_maps):
        per_core = [[np.asarray(m[name]) for name in in_names] for m in in_maps]
        concat_in = [
            np.concatenate([per_core[c][i] for c in range(N_CORES)], axis=0)
            for i in range(n_params)
        ]
        concat_zeros = [
            np.zeros((N_CORES * z.shape[0], *z.shape[1:]), z.dtype)
            for z in zero_outs
        ]
        out_arrs = sharded(*concat_in, *concat_zeros)
        return [
            {
                name: np.asarray(out_arrs[i]).reshape(
                    N_CORES, *out_avals[i].shape
                )[c]
                for i, name in enumerate(out_names)
            }
            for c in range(N_CORES)
        ]

    _CACHE["runner"] = (nc, run)
    return run


def kernel(**inputs) -> np.ndarray:
    x = np.asarray(inputs["x"], dtype=np.float32)
    wqk, wv, wout = _pack_weights(
        np.asarray(inputs["w_qkv"], dtype=np.float32),
        np.asarray(inputs["w_out"], dtype=np.float32),
    )
    common = {
        "w_qk": wqk,
        "w_v": wv,
        "w_out": wout,
        "b_qkv": np.asarray(inputs["b_qkv"], dtype=np.float32),
        "b_out": np.asarray(inputs["b_out"], dtype=np.float32),
        "gn_scale": np.asarray(inputs["gn_scale"], dtype=np.float32),
        "gn_bias": np.asarray(inputs["gn_bias"], dtype=np.float32),
    }
    in_maps = [
        {"x": np.ascontiguousarray(x[i].reshape(S, C)), **common} for i in range(B)
    ]
    bq = common["b_qkv"]
    nc = _get_program(
        has_bqk=bool(np.any(bq.reshape(NH, 3, HD)[:, 0:2, :])),
        has_bv=bool(np.any(bq.reshape(NH, 3, HD)[:, 2, :])),
        has_bout=bool(np.any(common["b_out"])),
    )
    res = _get_runner(nc)(in_maps)
    y = np.stack([r["y"] for r in res], axis=0)
    return y.reshape(B, 32, 32, C).astype(np.float32)


# revision 52
# speedup vs baseline: 1.0967x; 1.0481x over previous
"""Trainium2 Bass kernel for an AttentionBlock (GroupNorm + MHA + residual).

Shapes (hardcoded from the problem spec):
  x: [8, 32, 32, 512] fp32, t: [8] (unused by the reference),
  gn_scale/gn_bias: [512], w_qkv: [512, 1536], b_qkv: [1536],
  w_out: [512, 512], b_out: [512].

Sharding: pure data parallel — one batch element per NeuronCore (8 cores).
Each core runs the full block on its [1024, 512] slice; no collectives.

Per-core compute layout:
  - GroupNorm stats via ones-vector matmuls (sum over s on the PE),
    rsqrt as exp(-0.5*ln(var+eps)) so ScalarE stays in one table set.
  - xn transposed to [c, s] with PE transposes; qT/kT computed per head-pair
    [128, 1024] so QK^T row-packs two K=64 heads onto the PE array.
  - scores are computed transposed ([sk, sq]) so the softmax denominator and
    the attention*V contraction both come out of matmuls without transposing
    the [1024, 1024] attention matrix: AV uses v_aug=[v | 1] (M=65) stationary,
    and row 64 of the PSUM accumulator is the softmax row-sum.
  - exp runs on ScalarE straight out of PSUM, bf16 out.
  - matmul operands are bf16 (fp32 matmuls are 4x slower on TRN2); all
    accumulation is fp32 in PSUM; GroupNorm/normalize/residual math is fp32.

PSUM budget (8 banks, statically reserved by tile tags):
  psc [128,1024] f32 x1 = 2 banks (also used for the GN stats accumulator),
  acc [128,512] f32 x2 = 2 banks (transposes, qkv/out-proj accumulators),
  po  [128,512] f32 x4 = 4 banks (per-pair AV accumulators, 2 heads x 2 chunks).
"""

import numpy as np
import ml_dtypes

import concourse.bass as bass
import concourse.mybir as mybir
import concourse.tile as tile
from concourse.masks import make_identity


def _legalize_bir(bir_json: bytes) -> bytes:
    """This container's walrus encodes at most ONE sync wait per instruction
    ('Too many sync wait commands' otherwise), while the Tile scheduler freely
    emits 2+ (and the tail drain waits on every live semaphore). Legalize at
    the BIR level: for every instruction with N>1 waits, insert N-1 NoOps
    carrying one wait each immediately before it on the same engine — the
    engine executes them serially, so blocking semantics are identical."""
    import json

    m = json.loads(bir_json)
    n = 0
    for fn in m["functions"]:
        for bb in fn["blocks"]:
            out = []
            for inst in bb["instructions"]:
                si = inst.get("sync_info")
                waits = (si or {}).get("on_wait", [])
                if si is not None and len(waits) > 1:
                    for w in waits[:-1]:
                        n += 1
                        out.append({
                            "debug": inst.get("debug", 0),
                            "engine": inst["engine"],
                            "ins": [],
                            "outs": [],
                            "name": f"I-sqw-{n}",
                            "opcode": "NoOp",
                            "sync_info": {"on_update": [], "on_wait": [w]},
                        })
                    si["on_wait"] = [waits[-1]]
                out.append(inst)
            bb["instructions"] = out
    return json.dumps(m).encode()


def _install_bir_legalizer():
    import concourse.bass_utils as bu
    import concourse.bass2jax as b2j

    if getattr(bu, "_bir_legalizer_installed", False):
        return
    orig = bu.compile_bir_kernel

    def wrapped(bir_json, tmpdir, neff_name="file.neff"):
        return orig(_legalize_bir(bir_json), tmpdir, neff_name)

    bu.compile_bir_kernel = wrapped
    b2j.compile_bir_kernel = wrapped
    bu._bir_legalizer_installed = True


FP32 = mybir.dt.float32
BF16 = mybir.dt.bfloat16
AF = mybir.ActivationFunctionType
ALU = mybir.AluOpType

B = 8
S = 1024          # 32 * 32
C = 512
NH = 8
HD = 64
INNER = NH * HD   # 512
G = 32            # groups
GS = C // G       # 16 channels per group
EPS = 1e-5
N_CORES = 8

ST = S // 128     # 8 s-tiles
CT = C // 128     # 4 c-tiles
PAIRS = NH // 2   # 4 head pairs
SQC = S // 512    # 2 sq chunks of 512


def _pack_weights(wqkv_f32: np.ndarray, wout_f32: np.ndarray):
    """Host-side packing: w_qkv [C, 3*INNER] -> (w_qk [C, 1024], w_v [C, 512])
    so every matmul stationary slice is contiguous (1 free dim).

    w_qk layout: [c, pair, {q,k}, head_in_pair, d] flattened;
    w_v layout:  [c, head, d] flattened."""
    bf = ml_dtypes.bfloat16
    r = wqkv_f32.reshape(C, NH, 3, HD)
    wqk = (
        r[:, :, 0:2, :]
        .reshape(C, PAIRS, 2, 2, HD)  # [c, pair, h2, j, d]
        .transpose(0, 1, 3, 2, 4)     # [c, pair, j, h2, d]
        .reshape(C, PAIRS * 2 * 2 * HD)
        .astype(bf)
    )
    wv = r[:, :, 2, :].reshape(C, INNER).astype(bf)
    return np.ascontiguousarray(wqk), np.ascontiguousarray(wv), wout_f32.astype(bf)


def _dma(nc, out, in_):
    nc.sync.dma_start(out, in_)


def _emit(nc: bass.Bass, has_bqk: bool, has_bv: bool, has_bout: bool):
    x_d = nc.dram_tensor("x", [S, C], FP32, kind="ExternalInput")
    wqk_d = nc.dram_tensor("w_qk", [C, PAIRS * 2 * 128], BF16, kind="ExternalInput")
    wv_d = nc.dram_tensor("w_v", [C, INNER], BF16, kind="ExternalInput")
    wout_d = nc.dram_tensor("w_out", [INNER, C], BF16, kind="ExternalInput")
    bqkv_d = nc.dram_tensor("b_qkv", [3 * INNER], FP32, kind="ExternalInput")
    bout_d = nc.dram_tensor("b_out", [C], FP32, kind="ExternalInput")
    gns_d = nc.dram_tensor("gn_scale", [C], FP32, kind="ExternalInput")
    gnb_d = nc.dram_tensor("gn_bias", [C], FP32, kind="ExternalInput")
    y_d = nc.dram_tensor("y", [S, C], FP32, kind="ExternalOutput")
    gn_scratch = nc.dram_tensor("gn_scratch", [2, C], FP32, kind="Internal")

    with tile.TileContext(nc) as tc:
        with (
            tc.tile_pool(name="const", bufs=1) as const,
            tc.tile_pool(name="big", bufs=1) as big,
            tc.tile_pool(name="work", bufs=2) as work,
            tc.tile_pool(name="expp", bufs=3) as expp,
            tc.tile_pool(name="ps", bufs=1, space="PSUM") as ps,
        ):
            # ---------- constants / weights ----------
            ident = const.tile([128, 128], BF16)
            make_identity(nc, ident)
            ones_col = const.tile([128, 1], BF16)
            nc.vector.memset(ones_col, 1.0)
            eps_t = const.tile([1, 1], FP32)
            nc.vector.memset(eps_t, EPS)
            ones_r64 = const.tile([1, HD], BF16)
            nc.vector.memset(ones_r64, 1.0)
            ones_r128 = const.tile([1, 128], FP32)
            nc.vector.memset(ones_r128, 1.0)
            actwarm = const.tile([1, 2], FP32)
            nc.scalar.activation(actwarm[:, 0:1], eps_t, AF.Ln)
            nc.scalar.activation(actwarm[:, 1:2], eps_t, AF.Exp)

            # ---------- load x (per-tile DMAs for pipelining) ----------
            x_sb = big.tile([128, ST, C], FP32)
            x_t = x_d.rearrange("(t p) c -> t p c", p=128)
            for t in range(ST):
                _dma(nc, x_sb[:, t], x_t[t])

            wqk_sb = const.tile([128, CT, PAIRS, 2, 128], BF16)
            _dma(nc, 
                wqk_sb,
                wqk_d.rearrange("(ct p) (pr j f) -> p ct pr j f", p=128, pr=PAIRS, j=2),
            )
            wv_sb = const.tile([128, CT, INNER], BF16)
            _dma(nc, wv_sb, wv_d.rearrange("(ct p) f -> p ct f", p=128))
            wout_sb = const.tile([128, CT, C], BF16)
            _dma(nc, wout_sb, wout_d.rearrange("(ct p) f -> p ct f", p=128))
            gamma = const.tile([1, G, GS], FP32)
            _dma(nc, gamma, gns_d[None, :])
            beta = const.tile([1, G, GS], FP32)
            _dma(nc, beta, gnb_d[None, :])

            bqkv_hx = bqkv_d[None, :].rearrange("o (h x) -> o h x", x=3 * HD)
            bqk = bv_bc = bout_bc = None
            if has_bqk:
                # per-partition bias columns for the q/k projections per pair
                bqk = const.tile([128, PAIRS, 2], FP32)
                for p in range(PAIRS):
                    for j in range(2):  # 0 -> q cols, 1 -> k cols
                        _dma(nc,
                            bqk[:, p, j : j + 1],
                            bqkv_hx[0, 2 * p : 2 * p + 2, j * HD : (j + 1) * HD],
                        )
            if has_bv:
                # free-dim bias vector broadcast to 128 partitions via DMA
                # (DRAM-side partition broadcast is allowed, SBUF-side is not)
                bv_bc = const.tile([128, NH, HD], FP32)
                _dma(nc,
                    bv_bc, bqkv_hx[:, :, 2 * HD : 3 * HD].partition_broadcast(128)
                )
            if has_bout:
                bout_bc = const.tile([128, C], FP32)
                _dma(nc, bout_bc, bout_d[None, :].partition_broadcast(128))

            # ---------- GroupNorm stats (the critical chain: emit first so the
            # stats -> scale/bias pipeline outranks the transpose evacuations)
            xbf = big.tile([128, ST, C], BF16)
            xT = [
                big.tile([128, S], BF16, tag=f"xT{ct}", name=f"xT{ct}")
                for ct in range(CT)
            ]
            pstat = ps.tile([1, 2, C], FP32, tag="big2", bufs=3)
            for t in range(ST):
                nc.gpsimd.tensor_copy(xbf[:, t], x_sb[:, t])
                xsq_t = work.tile([128, C], BF16, tag="xsq")
                nc.scalar.activation(xsq_t, xbf[:, t], AF.Square)
                nc.tensor.matmul(
                    pstat[:, 0], ones_col, xbf[:, t], start=(t == 0), stop=(t == ST - 1)
                )
                nc.tensor.matmul(
                    pstat[:, 1], ones_col, xsq_t, start=(t == 0), stop=(t == ST - 1)
                )

            gstat = work.tile([1, 2, G], FP32, tag="gstat")
            nc.vector.reduce_sum(
                gstat,
                pstat.rearrange("p s (g d) -> p s g d", d=GS),
                axis=mybir.AxisListType.X,
            )
            inv_n = 1.0 / (S * GS)
            mean_g = work.tile([1, G], FP32, tag="mean_g")
            nc.vector.tensor_scalar_mul(mean_g, gstat[:, 0], inv_n)
            var_g = work.tile([1, G], FP32, tag="var_g")
            nc.vector.tensor_scalar_mul(var_g, gstat[:, 1], inv_n)
            msq = work.tile([1, G], FP32, tag="msq")
            nc.vector.tensor_mul(msq, mean_g, mean_g)
            nc.vector.tensor_sub(var_g, var_g, msq)
            # rstd = exp(-0.5 * ln(var + eps))  (stays in the exp table set)
            lnv = work.tile([1, G], FP32, tag="lnv")
            nc.scalar.activation(lnv, var_g, AF.Ln, bias=eps_t)
            rstd_g = work.tile([1, G], FP32, tag="rstd_g")
            nc.scalar.activation(rstd_g, lnv, AF.Exp, scale=-0.5)

            # sb_c[0] = scale_c = rstd_g * gamma ; sb_c[1] = bias_c
            sb_c = work.tile([1, 2, G, GS], FP32, tag="sb_c")
            scale_c = sb_c[:, 0]
            bias_c = sb_c[:, 1]
            nc.vector.tensor_tensor(
                scale_c, gamma, rstd_g[:, :, None].to_broadcast([1, G, GS]), ALU.mult
            )
            nc.vector.tensor_tensor(
                bias_c, scale_c, mean_g[:, :, None].to_broadcast([1, G, GS]), ALU.mult
            )
            nc.vector.tensor_sub(bias_c, beta, bias_c)

            # per-partition (channel-major) copies of scale/bias for the
            # transposed-side GN application on ScalarE (DRAM bounce: DMA
            # cannot scatter SBUF free dim -> partitions directly)
            _dma(nc, gn_scratch[None, :], sb_c.rearrange("o s g d -> o s (g d)"))
            sbp = const.tile([128, 2, CT], FP32)
            _dma(nc, sbp, gn_scratch.rearrange("s (ct p) -> p s ct", p=128))
            scale_cp = sbp[:, 0]
            bias_cp = sbp[:, 1]
            # broadcast copies for the natural-layout residual xn: K=1
            # ones-matmuls (a stride-0 partition-broadcast DMA runs at ~2GB/s)
            sb_bc = ps.tile([128, 2, C], FP32, tag="big2", bufs=3, name="sb_bc")
            nc.tensor.matmul(
                sb_bc[:, 0], ones_r128, sb_c.rearrange("o s g d -> o (s g d)")[:, 0:C],
                start=True, stop=True,
            )
            nc.tensor.matmul(
                sb_bc[:, 1], ones_r128,
                sb_c.rearrange("o s g d -> o (s g d)")[:, C : 2 * C],
                start=True, stop=True,
            )
            scale_bc = sb_bc[:, 0]
            bias_bc = sb_bc[:, 1]

            # transposes of raw x (independent of the stats chain, lower prio)
            for t in range(ST):
                for ct in range(CT):
                    ptr = ps.tile([128, 128], BF16, tag="acc", bufs=2)
                    nc.tensor.transpose(
                        ptr, xbf[:, t, 128 * ct : 128 * (ct + 1)], ident
                    )
                    nc.vector.tensor_copy(xT[ct][:, 128 * t : 128 * (t + 1)], ptr)

            # ---------- GN on the transposed side: xnT = xT*scale + bias ----
            xnT = [
                big.tile([128, S], BF16, tag=f"xnT{ct}", name=f"xnT{ct}")
                for ct in range(CT)
            ]
            for ct in range(CT):
                nc.scalar.activation(
                    xnT[ct],
                    xT[ct],
                    AF.Identity,
                    bias=bias_cp[:, ct : ct + 1],
                    scale=scale_cp[:, ct : ct + 1],
                )

            # ---------- per head pair: q/k projections, then attention ----------
            # outT is split per (pair, sq-chunk) so the output projection can
            # start as soon as the chunk it reads is normalized.
            outT = [
                [
                    big.tile(
                        [128, 512], BF16, tag=f"outT{p}_{cc}", name=f"outT{p}_{cc}"
                    )
                    for cc in range(SQC)
                ]
                for p in range(PAIRS)
            ]

            def alloc_qk(p):
                qT = work.tile([128, S], BF16, tag="qT", name=f"qT{p}")
                kT = work.tile([128, S], BF16, tag="kT", name=f"kT{p}")
                return qT, kT

            def emit_qk_group(p, qT, kT, j, cchunk):
                dst = (qT, kT)[j]
                pqk = ps.tile([128, 512], FP32, tag="acc", bufs=2)
                for ct in range(CT):
                    nc.tensor.matmul(
                        pqk,
                        wqk_sb[:, ct, p, j],
                        xnT[ct][:, 512 * cchunk : 512 * (cchunk + 1)],
                        start=(ct == 0),
                        stop=(ct == CT - 1),
                    )
                if has_bqk:
                    nc.vector.tensor_scalar(
                        dst[:, 512 * cchunk : 512 * (cchunk + 1)],
                        pqk,
                        bqk[:, p, j : j + 1],
                        None,
                        ALU.add,
                    )
                else:
                    nc.vector.tensor_copy(
                        dst[:, 512 * cchunk : 512 * (cchunk + 1)], pqk
                    )

            qk_tiles = {0: alloc_qk(0)}
            for j in (0, 1):
                for cc in range(SQC):
                    emit_qk_group(0, *qk_tiles[0], j, cc)

            # ---------- v projection (natural layout) + ones column ----------
            # emitted lazily inside pair-0/head-0's loop so the first QK
            # matmuls outrank it in the PE's static order
            v_sb = big.tile([128, ST, NH, HD + 1], BF16)

            def emit_v(t):
                pv = ps.tile([128, INNER], FP32, tag="acc", bufs=2)
                for ct in range(CT):
                    nc.tensor.matmul(
                        pv,
                        xnT[ct][:, 128 * t : 128 * (t + 1)],
                        wv_sb[:, ct],
                        start=(ct == 0),
                        stop=(ct == CT - 1),
                    )
                if has_bv:
                    nc.vector.tensor_tensor(
                        v_sb[:, t, :, 0:HD],
                        pv.rearrange("p (h d) -> p h d", d=HD),
                        bv_bc,
                        ALU.add,
                    )
                else:
                    nc.vector.tensor_copy(
                        v_sb[:, t, :, 0:HD], pv.rearrange("p (h d) -> p h d", d=HD)
                    )
                nc.gpsimd.memset(v_sb[:, t, :, HD : HD + 1], 1.0)

            # ---------- natural-layout xn for the residual (off critical path)
            sb_bc_sb = big.tile([128, 2, C], FP32)
            nc.vector.tensor_copy(sb_bc_sb, sb_bc)
            xn = big.tile([128, ST, C], FP32)
            for t in range(ST):
                nc.vector.tensor_tensor(
                    xn[:, t], x_sb[:, t], sb_bc_sb[:, 0], ALU.mult
                )
                nc.vector.tensor_tensor(
                    xn[:, t], xn[:, t], sb_bc_sb[:, 1], ALU.add
                )

            for p in range(PAIRS):
                qT, kT = qk_tiles.pop(p)
                if p + 1 < PAIRS:
                    qk_tiles[p + 1] = alloc_qk(p + 1)
                    next_groups = [(j, cc) for j in (0, 1) for cc in range(SQC)]
                # attention, one head at a time. The PE is in-order, so AV
                # matmuls (which wait on exp) are emitted with a 2-step lag
                # behind the QK matmuls: the exp chain on ScalarE then never
                # waits for the PE.
                for h in range(2):
                    po_h = ps.tile(
                        [HD + 1, SQC, 512], FP32, tag="big2", bufs=3, name=f"po{p}_{h}"
                    )

                    def emit_av(skt, et):
                        for cc in range(SQC):
                            nc.tensor.matmul(
                                po_h[:, cc, :],
                                v_sb[:, skt, 2 * p + h],
                                et[:, 512 * cc : 512 * (cc + 1)],
                                start=(skt == 0),
                                stop=(skt == ST - 1),
                            )

                    pend = []
                    for skt in range(ST):
                        psc = ps.tile([128, S], FP32, tag="big2", bufs=3)
                        for cc in range(SQC):
                            nc.tensor.matmul(
                                psc[:, 512 * cc : 512 * (cc + 1)],
                                kT[64 * h : 64 * (h + 1), 128 * skt : 128 * (skt + 1)],
                                qT[64 * h : 64 * (h + 1), 512 * cc : 512 * (cc + 1)],
                                start=True,
                                stop=True,
                            )
                        et = expp.tile([128, S], BF16, tag="exp", bufs=12)
                        nc.scalar.activation(et, psc, AF.Exp, scale=1.0 / 8.0)
                        if p == 0 and h == 0:
                            emit_v(skt)
                        if h == 1 and p + 1 < PAIRS and skt % 2 == 0:
                            j, cc = next_groups[skt // 2]
                            emit_qk_group(p + 1, *qk_tiles[p + 1], j, cc)
                        pend.append((skt, et))
                        if len(pend) > 4:
                            emit_av(*pend.pop(0))
                    for item in pend:
                        emit_av(*item)
                    # normalize: out / rowsum (rowsum is PSUM row 64).
                    # 1/r = exp(-ln(r)) on ScalarE: a 1-lane DVE reciprocal is
                    # ~8 cycles/element (4us per row on HW) while ACT runs
                    # 1 cycle/element and reads PSUM directly. The fp32
                    # reciprocal row is then broadcast across the 64 head-dim
                    # partitions with a stride-0 SBUF->SBUF DMA.
                    lnr = work.tile([1, SQC, 512], FP32, tag="lnr")
                    nc.scalar.activation(lnr, po_h[HD : HD + 1, :, :], AF.Ln)
                    recip = work.tile([1, SQC, 512], BF16, tag="recip")
                    nc.scalar.activation(recip, lnr, AF.Exp, scale=-1.0)
                    pou = work.tile([HD, SQC, 512], FP32, tag="pou")
                    nc.vector.tensor_copy(pou, po_h[0:HD, :, :])
                    for cc in range(SQC):
                        rbc = ps.tile(
                            [HD, 512], FP32, tag="acc", bufs=2, name=f"rbc{h}_{cc}"
                        )
                        nc.tensor.matmul(
                            rbc, ones_r64, recip[:, cc], start=True, stop=True
                        )
                        nc.vector.tensor_tensor(
                            outT[p][cc][64 * h : 64 * (h + 1), :],
                            pou[:, cc, :],
                            rbc,
                            ALU.mult,
                        )

            # ---------- output projection + residual ----------
            for t in range(ST):
                ph = ps.tile([128, C], FP32, tag="acc", bufs=2)
                for p in range(PAIRS):
                    nc.tensor.matmul(
                        ph,
                        outT[p][t // 4][:, 128 * (t % 4) : 128 * (t % 4 + 1)],
                        wout_sb[:, p],
                        start=(p == 0),
                        stop=(p == PAIRS - 1),
                    )
                yt = work.tile([128, C], FP32, tag="y")
                nc.vector.tensor_tensor(yt, ph, xn[:, t], ALU.add)
                if has_bout:
                    nc.vector.tensor_tensor(yt, yt, bout_bc, ALU.add)
                _dma(nc, y_d[128 * t : 128 * (t + 1), :], yt)

    return nc


_CACHE: dict = {}


def _get_program(has_bqk: bool = True, has_bv: bool = True, has_bout: bool = True) -> bass.Bass:
    key = (has_bqk, has_bv, has_bout)
    if key not in _CACHE:
        _install_bir_legalizer()
        nc = bass.Bass()
        _emit(nc, has_bqk, has_bv, has_bout)
        _CACHE[key] = nc
    return _CACHE[key]


def _get_runner(nc: bass.Bass):
    """Build the sharded PJRT executable once and cache it — bass2jax's
    run_bass_via_pjrt re-jits a fresh closure every call (~1.5 s of host-side
    retrace per invocation)."""
    if "runner" in _CACHE and _CACHE["runner"][0] is nc:
        return _CACHE["runner"][1]

    import jax
    from jax.sharding import Mesh, PartitionSpec
    from jax.experimental.shard_map import shard_map
    from concourse import bass2jax, mybir as _mb

    bass2jax.install_neuronx_cc_hook()
    partition_name = (
        nc.partition_id_tensor.name if nc.partition_id_tensor else None
    )
    in_names, out_names, out_avals, zero_outs = [], [], [], []
    for alloc in nc.m.functions[0].allocations:
        if not isinstance(alloc, _mb.MemoryLocationSet):
            continue
        name = alloc.memorylocations[0].name
        if alloc.kind == "ExternalInput":
            if name != partition_name:
                in_names.append(name)
        elif alloc.kind == "ExternalOutput":
            shape = tuple(alloc.tensor_shape)
            dtype = _mb.dt.np(alloc.dtype)
            out_names.append(name)
            out_avals.append(jax.core.ShapedArray(shape, dtype))
            zero_outs.append(np.zeros(shape, dtype))
    n_params = len(in_names)
    n_outs = len(out_avals)
    all_in_names = list(in_names) + list(out_names)
    if partition_name is not None:
        all_in_names.append(partition_name)
    donate = tuple(range(n_params, n_params + n_outs))

    def _body(*args):
        operands = list(args)
        if partition_name is not None:
            operands.append(bass2jax.partition_id_tensor())
        outs = bass2jax._bass_exec_p.bind(
            *operands,
            out_avals=tuple(out_avals),
            in_names=tuple(all_in_names),
            out_names=tuple(out_names),
            lowering_input_output_aliases=(),
            sim_require_finite=True,
            sim_require_nnan=True,
            nc=nc,
        )
        return tuple(outs)

    devices = jax.devices()[:N_CORES]
    mesh = Mesh(np.asarray(devices), ("core",))
    in_specs = (PartitionSpec("core"),) * (n_params + n_outs)
    out_specs = (PartitionSpec("core"),) * n_outs
    sharded = jax.jit(
        shard_map(
            _body, mesh=mesh, in_specs=in_specs, out_specs=out_specs,
            check_rep=False,
        ),
        donate_argnums=donate,
        keep_unused=True,
    )

    def run(in_maps):
        per_core = [[np.asarray(m[name]) for name in in_names] for m in in_maps]
        concat_in = [
            np.concatenate([per_core[c][i] for c in range(N_CORES)], axis=0)
            for i in range(n_params)
        ]
        concat_zeros = [
            np.zeros((N_CORES * z.shape[0], *z.shape[1:]), z.dtype)
            for z in zero_outs
        ]
        out_arrs = sharded(*concat_in, *concat_zeros)
        return [
            {
                name: np.asarray(out_arrs[i]).reshape(
                    N_CORES, *out_avals[i].shape
                )[c]
                for i, name in enumerate(out_names)
            }
            for c in range(N_CORES)
        ]

    _CACHE["runner"] = (nc, run)
    return run


def kernel(**inputs) -> np.ndarray:
    x = np.asarray(inputs["x"], dtype=np.float32)
    wqk, wv, wout = _pack_weights(
        np.asarray(inputs["w_qkv"], dtype=np.float32),
        np.asarray(inputs["w_out"], dtype=np.float32),
    )
    common = {
        "w_qk": wqk,
        "w_v": wv,
        "w_out": wout,
        "b_qkv": np.asarray(inputs["b_qkv"], dtype=np.float32),
        "b_out": np.asarray(inputs["b_out"], dtype=np.float32),
        "gn_scale": np.asarray(inputs["gn_scale"], dtype=np.float32),
        "gn_bias": np.asarray(inputs["gn_bias"], dtype=np.float32),
    }
    in_maps = [
        {"x": np.ascontiguousarray(x[i].reshape(S, C)), **common} for i in range(B)
    ]
    bq = common["b_qkv"]
    nc = _get_program(
        has_bqk=bool(np.any(bq.reshape(NH, 3, HD)[:, 0:2, :])),
        has_bv=bool(np.any(bq.reshape(NH, 3, HD)[:, 2, :])),
        has_bout=bool(np.any(common["b_out"])),
    )
    res = _get_runner(nc)(in_maps)
    y = np.stack([r["y"] for r in res], axis=0)
    return y.reshape(B, 32, 32, C).astype(np.float32)


# revision 53
# speedup vs baseline: 1.1945x; 1.0891x over previous
"""Trainium2 Bass kernel for an AttentionBlock (GroupNorm + MHA + residual).

Shapes (hardcoded from the problem spec):
  x: [8, 32, 32, 512] fp32, t: [8] (unused by the reference),
  gn_scale/gn_bias: [512], w_qkv: [512, 1536], b_qkv: [1536],
  w_out: [512, 512], b_out: [512].

Sharding: pure data parallel — one batch element per NeuronCore (8 cores).
Each core runs the full block on its [1024, 512] slice; no collectives.

Per-core compute layout:
  - GroupNorm stats via ones-vector matmuls (sum over s on the PE),
    rsqrt as exp(-0.5*ln(var+eps)) so ScalarE stays in one table set.
  - xn transposed to [c, s] with PE transposes; qT/kT computed per head-pair
    [128, 1024] so QK^T row-packs two K=64 heads onto the PE array.
  - scores are computed transposed ([sk, sq]) so the softmax denominator and
    the attention*V contraction both come out of matmuls without transposing
    the [1024, 1024] attention matrix: AV uses v_aug=[v | 1] (M=65) stationary,
    and row 64 of the PSUM accumulator is the softmax row-sum.
  - exp runs on ScalarE straight out of PSUM, bf16 out.
  - matmul operands are bf16 (fp32 matmuls are 4x slower on TRN2); all
    accumulation is fp32 in PSUM; GroupNorm/normalize/residual math is fp32.

PSUM budget (8 banks, statically reserved by tile tags):
  psc [128,1024] f32 x1 = 2 banks (also used for the GN stats accumulator),
  acc [128,512] f32 x2 = 2 banks (transposes, qkv/out-proj accumulators),
  po  [128,512] f32 x4 = 4 banks (per-pair AV accumulators, 2 heads x 2 chunks).
"""

import numpy as np
import ml_dtypes

import concourse.bass as bass
import concourse.mybir as mybir
import concourse.tile as tile
from concourse.masks import make_identity


def _legalize_bir(bir_json: bytes) -> bytes:
    """This container's walrus encodes at most ONE sync wait per instruction
    ('Too many sync wait commands' otherwise), while the Tile scheduler freely
    emits 2+ (and the tail drain waits on every live semaphore). Legalize at
    the BIR level: for every instruction with N>1 waits, insert N-1 NoOps
    carrying one wait each immediately before it on the same engine — the
    engine executes them serially, so blocking semantics are identical."""
    import json

    m = json.loads(bir_json)
    n = 0
    for fn in m["functions"]:
        for bb in fn["blocks"]:
            out = []
            for inst in bb["instructions"]:
                si = inst.get("sync_info")
                waits = (si or {}).get("on_wait", [])
                if si is not None and len(waits) > 1:
                    for w in waits[:-1]:
                        n += 1
                        out.append({
                            "debug": inst.get("debug", 0),
                            "engine": inst["engine"],
                            "ins": [],
                            "outs": [],
                            "name": f"I-sqw-{n}",
                            "opcode": "NoOp",
                            "sync_info": {"on_update": [], "on_wait": [w]},
                        })
                    si["on_wait"] = [waits[-1]]
                out.append(inst)
            bb["instructions"] = out
    return json.dumps(m).encode()


def _install_bir_legalizer():
    import concourse.bass_utils as bu
    import concourse.bass2jax as b2j

    if getattr(bu, "_bir_legalizer_installed", False):
        return
    orig = bu.compile_bir_kernel

    def wrapped(bir_json, tmpdir, neff_name="file.neff"):
        return orig(_legalize_bir(bir_json), tmpdir, neff_name)

    bu.compile_bir_kernel = wrapped
    b2j.compile_bir_kernel = wrapped
    bu._bir_legalizer_installed = True


FP32 = mybir.dt.float32
BF16 = mybir.dt.bfloat16
AF = mybir.ActivationFunctionType
ALU = mybir.AluOpType

B = 8
S = 1024          # 32 * 32
C = 512
NH = 8
HD = 64
INNER = NH * HD   # 512
G = 32            # groups
GS = C // G       # 16 channels per group
EPS = 1e-5
N_CORES = 8

ST = S // 128     # 8 s-tiles
CT = C // 128     # 4 c-tiles
PAIRS = NH // 2   # 4 head pairs
SQC = S // 512    # 2 sq chunks of 512


def _pack_weights(wqkv_f32: np.ndarray, wout_f32: np.ndarray):
    """Host-side packing: w_qkv [C, 3*INNER] -> (w_qk [C, 1024], w_v [C, 512])
    so every matmul stationary slice is contiguous (1 free dim).

    w_qk layout: [c, pair, {q,k}, head_in_pair, d] flattened;
    w_v layout:  [c, head, d] flattened."""
    bf = ml_dtypes.bfloat16
    r = wqkv_f32.reshape(C, NH, 3, HD)
    wqk = (
        r[:, :, 0:2, :]
        .reshape(C, PAIRS, 2, 2, HD)  # [c, pair, h2, j, d]
        .transpose(0, 1, 3, 2, 4)     # [c, pair, j, h2, d]
        .reshape(C, PAIRS * 2 * 2 * HD)
        .astype(bf)
    )
    wv = r[:, :, 2, :].reshape(C, INNER).astype(bf)
    return np.ascontiguousarray(wqk), np.ascontiguousarray(wv), wout_f32.astype(bf)


def _dma(nc, out, in_):
    nc.sync.dma_start(out, in_)


def _emit(nc: bass.Bass, has_bqk: bool, has_bv: bool, has_bout: bool):
    x_d = nc.dram_tensor("x", [S, C], FP32, kind="ExternalInput")
    wqk_d = nc.dram_tensor("w_qk", [C, PAIRS * 2 * 128], BF16, kind="ExternalInput")
    wv_d = nc.dram_tensor("w_v", [C, INNER], BF16, kind="ExternalInput")
    wout_d = nc.dram_tensor("w_out", [INNER, C], BF16, kind="ExternalInput")
    bqkv_d = nc.dram_tensor("b_qkv", [3 * INNER], FP32, kind="ExternalInput")
    bout_d = nc.dram_tensor("b_out", [C], FP32, kind="ExternalInput")
    gns_d = nc.dram_tensor("gn_scale", [C], FP32, kind="ExternalInput")
    gnb_d = nc.dram_tensor("gn_bias", [C], FP32, kind="ExternalInput")
    y_d = nc.dram_tensor("y", [S, C], FP32, kind="ExternalOutput")
    gn_scratch = nc.dram_tensor("gn_scratch", [2, C], FP32, kind="Internal")

    with tile.TileContext(nc) as tc:
        with (
            tc.tile_pool(name="const", bufs=1) as const,
            tc.tile_pool(name="big", bufs=1) as big,
            tc.tile_pool(name="work", bufs=2) as work,
            tc.tile_pool(name="expp", bufs=3) as expp,
            tc.tile_pool(name="ps", bufs=1, space="PSUM") as ps,
        ):
            # ---------- constants / weights ----------
            ident = const.tile([128, 128], BF16)
            make_identity(nc, ident)
            ones_col = const.tile([128, 1], BF16)
            nc.vector.memset(ones_col, 1.0)
            eps_t = const.tile([1, 1], FP32)
            nc.vector.memset(eps_t, EPS)
            ones_r64 = const.tile([1, HD], BF16)
            nc.vector.memset(ones_r64, 1.0)
            ones_r128 = const.tile([1, 128], FP32)
            nc.vector.memset(ones_r128, 1.0)
            actwarm = const.tile([1, 2], FP32)
            nc.scalar.activation(actwarm[:, 0:1], eps_t, AF.Ln)
            nc.scalar.activation(actwarm[:, 1:2], eps_t, AF.Exp)

            # ---------- load x (per-tile DMAs for pipelining) ----------
            x_sb = big.tile([128, ST, C], FP32)
            x_t = x_d.rearrange("(t p) c -> t p c", p=128)
            for t in range(ST):
                _dma(nc, x_sb[:, t], x_t[t])

            wqk_sb = const.tile([128, CT, PAIRS, 2, 128], BF16)
            _dma(nc, 
                wqk_sb,
                wqk_d.rearrange("(ct p) (pr j f) -> p ct pr j f", p=128, pr=PAIRS, j=2),
            )
            wv_sb = const.tile([128, CT, INNER], BF16)
            _dma(nc, wv_sb, wv_d.rearrange("(ct p) f -> p ct f", p=128))
            wout_sb = const.tile([128, CT, C], BF16)
            _dma(nc, wout_sb, wout_d.rearrange("(ct p) f -> p ct f", p=128))
            gamma = const.tile([1, G, GS], FP32)
            _dma(nc, gamma, gns_d[None, :])
            beta = const.tile([1, G, GS], FP32)
            _dma(nc, beta, gnb_d[None, :])

            bqkv_hx = bqkv_d[None, :].rearrange("o (h x) -> o h x", x=3 * HD)
            bqk = bv_bc = bout_bc = None
            if has_bqk:
                # per-partition bias columns for the q/k projections per pair
                bqk = const.tile([128, PAIRS, 2], FP32)
                for p in range(PAIRS):
                    for j in range(2):  # 0 -> q cols, 1 -> k cols
                        _dma(nc,
                            bqk[:, p, j : j + 1],
                            bqkv_hx[0, 2 * p : 2 * p + 2, j * HD : (j + 1) * HD],
                        )
            if has_bv:
                # free-dim bias vector broadcast to 128 partitions via DMA
                # (DRAM-side partition broadcast is allowed, SBUF-side is not)
                bv_bc = const.tile([128, NH, HD], FP32)
                _dma(nc,
                    bv_bc, bqkv_hx[:, :, 2 * HD : 3 * HD].partition_broadcast(128)
                )
            if has_bout:
                bout_bc = const.tile([128, C], FP32)
                _dma(nc, bout_bc, bout_d[None, :].partition_broadcast(128))

            # ---------- GroupNorm stats (the critical chain: emit first so the
            # stats -> scale/bias pipeline outranks the transpose evacuations)
            xbf = big.tile([128, ST, C], BF16)
            xT = [
                big.tile([128, S], BF16, tag=f"xT{ct}", name=f"xT{ct}")
                for ct in range(CT)
            ]
            pstat = ps.tile([1, 2, C], FP32, tag="big2", bufs=3)
            for t in range(ST):
                nc.vector.tensor_copy(xbf[:, t], x_sb[:, t])
                xsq_t = work.tile([128, C], BF16, tag="xsq")
                nc.scalar.activation(xsq_t, xbf[:, t], AF.Square)
                nc.tensor.matmul(
                    pstat[:, 0], ones_col, xbf[:, t], start=(t == 0), stop=(t == ST - 1)
                )
                nc.tensor.matmul(
                    pstat[:, 1], ones_col, xsq_t, start=(t == 0), stop=(t == ST - 1)
                )

            gstat = work.tile([1, 2, G], FP32, tag="gstat")
            nc.vector.reduce_sum(
                gstat,
                pstat.rearrange("p s (g d) -> p s g d", d=GS),
                axis=mybir.AxisListType.X,
            )
            inv_n = 1.0 / (S * GS)
            mean_g = work.tile([1, G], FP32, tag="mean_g")
            nc.vector.tensor_scalar_mul(mean_g, gstat[:, 0], inv_n)
            var_g = work.tile([1, G], FP32, tag="var_g")
            nc.vector.tensor_scalar_mul(var_g, gstat[:, 1], inv_n)
            msq = work.tile([1, G], FP32, tag="msq")
            nc.vector.tensor_mul(msq, mean_g, mean_g)
            nc.vector.tensor_sub(var_g, var_g, msq)
            # rstd = exp(-0.5 * ln(var + eps))  (stays in the exp table set)
            lnv = work.tile([1, G], FP32, tag="lnv")
            nc.scalar.activation(lnv, var_g, AF.Ln, bias=eps_t)
            rstd_g = work.tile([1, G], FP32, tag="rstd_g")
            nc.scalar.activation(rstd_g, lnv, AF.Exp, scale=-0.5)

            # sb_c[0] = scale_c = rstd_g * gamma ; sb_c[1] = bias_c
            sb_c = work.tile([1, 2, G, GS], FP32, tag="sb_c")
            scale_c = sb_c[:, 0]
            bias_c = sb_c[:, 1]
            nc.vector.tensor_tensor(
                scale_c, gamma, rstd_g[:, :, None].to_broadcast([1, G, GS]), ALU.mult
            )
            nc.vector.tensor_tensor(
                bias_c, scale_c, mean_g[:, :, None].to_broadcast([1, G, GS]), ALU.mult
            )
            nc.vector.tensor_sub(bias_c, beta, bias_c)

            # per-partition (channel-major) copies of scale/bias for the
            # transposed-side GN application on ScalarE (DRAM bounce: DMA
            # cannot scatter SBUF free dim -> partitions directly)
            _dma(nc, gn_scratch[None, :], sb_c.rearrange("o s g d -> o s (g d)"))
            sbp = const.tile([128, 2, CT], FP32)
            _dma(nc, sbp, gn_scratch.rearrange("s (ct p) -> p s ct", p=128))
            scale_cp = sbp[:, 0]
            bias_cp = sbp[:, 1]
            # broadcast copies for the natural-layout residual xn: K=1
            # ones-matmuls (a stride-0 partition-broadcast DMA runs at ~2GB/s)
            sb_bc = ps.tile([128, 2, C], FP32, tag="big2", bufs=3, name="sb_bc")
            nc.tensor.matmul(
                sb_bc[:, 0], ones_r128, sb_c.rearrange("o s g d -> o (s g d)")[:, 0:C],
                start=True, stop=True,
            )
            nc.tensor.matmul(
                sb_bc[:, 1], ones_r128,
                sb_c.rearrange("o s g d -> o (s g d)")[:, C : 2 * C],
                start=True, stop=True,
            )
            scale_bc = sb_bc[:, 0]
            bias_bc = sb_bc[:, 1]

            # transposes of raw x (independent of the stats chain, lower prio)
            for t in range(ST):
                for ct in range(CT):
                    ptr = ps.tile([128, 128], BF16, tag="acc", bufs=2)
                    nc.tensor.transpose(
                        ptr, xbf[:, t, 128 * ct : 128 * (ct + 1)], ident
                    )
                    nc.vector.tensor_copy(xT[ct][:, 128 * t : 128 * (t + 1)], ptr)

            # ---------- GN on the transposed side: xnT = xT*scale + bias ----
            xnT = [
                big.tile([128, S], BF16, tag=f"xnT{ct}", name=f"xnT{ct}")
                for ct in range(CT)
            ]
            for ct in range(CT):
                nc.scalar.activation(
                    xnT[ct],
                    xT[ct],
                    AF.Identity,
                    bias=bias_cp[:, ct : ct + 1],
                    scale=scale_cp[:, ct : ct + 1],
                )

            # ---------- per head pair: q/k projections, then attention ----------
            # outT is split per (pair, sq-chunk) so the output projection can
            # start as soon as the chunk it reads is normalized.
            outT = [
                [
                    big.tile(
                        [128, 512], BF16, tag=f"outT{p}_{cc}", name=f"outT{p}_{cc}"
                    )
                    for cc in range(SQC)
                ]
                for p in range(PAIRS)
            ]

            def alloc_qk(p):
                qT = work.tile([128, S], BF16, tag="qT", name=f"qT{p}")
                kT = work.tile([128, S], BF16, tag="kT", name=f"kT{p}")
                return qT, kT

            def emit_qk_group(p, qT, kT, j, cchunk):
                dst = (qT, kT)[j]
                pqk = ps.tile([128, 512], FP32, tag="acc", bufs=2)
                for ct in range(CT):
                    nc.tensor.matmul(
                        pqk,
                        wqk_sb[:, ct, p, j],
                        xnT[ct][:, 512 * cchunk : 512 * (cchunk + 1)],
                        start=(ct == 0),
                        stop=(ct == CT - 1),
                    )
                if has_bqk:
                    nc.vector.tensor_scalar(
                        dst[:, 512 * cchunk : 512 * (cchunk + 1)],
                        pqk,
                        bqk[:, p, j : j + 1],
                        None,
                        ALU.add,
                    )
                else:
                    nc.vector.tensor_copy(
                        dst[:, 512 * cchunk : 512 * (cchunk + 1)], pqk
                    )

            qk_tiles = {0: alloc_qk(0)}
            for j, cc in ((0, 0), (0, 1), (1, 0), (1, 1)):
                emit_qk_group(0, *qk_tiles[0], j, cc)

            # ---------- v projection (natural layout) + ones column ----------
            # emitted lazily inside pair-0/head-0's loop so the first QK
            # matmuls outrank it in the PE's static order
            v_sb = big.tile([128, ST, NH, HD + 1], BF16)

            def emit_v(t):
                pv = ps.tile([128, INNER], FP32, tag="acc", bufs=2)
                for ct in range(CT):
                    nc.tensor.matmul(
                        pv,
                        xnT[ct][:, 128 * t : 128 * (t + 1)],
                        wv_sb[:, ct],
                        start=(ct == 0),
                        stop=(ct == CT - 1),
                    )
                if has_bv:
                    nc.vector.tensor_tensor(
                        v_sb[:, t, :, 0:HD],
                        pv.rearrange("p (h d) -> p h d", d=HD),
                        bv_bc,
                        ALU.add,
                    )
                else:
                    nc.vector.tensor_copy(
                        v_sb[:, t, :, 0:HD], pv.rearrange("p (h d) -> p h d", d=HD)
                    )
                nc.gpsimd.memset(v_sb[:, t, :, HD : HD + 1], 1.0)

            # ---------- natural-layout xn for the residual (off critical path)
            sb_bc_sb = big.tile([128, 2, C], FP32)
            nc.vector.tensor_copy(sb_bc_sb, sb_bc)
            xn = big.tile([128, ST, C], FP32)
            for t in range(ST):
                nc.vector.tensor_tensor(
                    xn[:, t], x_sb[:, t], sb_bc_sb[:, 0], ALU.mult
                )
                nc.vector.tensor_tensor(
                    xn[:, t], xn[:, t], sb_bc_sb[:, 1], ALU.add
                )

            for p in range(PAIRS):
                qT, kT = qk_tiles.pop(p)
                if p + 1 < PAIRS:
                    qk_tiles[p + 1] = alloc_qk(p + 1)
                    next_groups = [(j, cc) for j in (0, 1) for cc in range(SQC)]
                # attention, one head at a time. The PE is in-order, so AV
                # matmuls (which wait on exp) are emitted with a 2-step lag
                # behind the QK matmuls: the exp chain on ScalarE then never
                # waits for the PE.
                for h in range(2):
                    po_h = ps.tile(
                        [HD + 1, SQC, 512], FP32, tag="big2", bufs=3, name=f"po{p}_{h}"
                    )

                    def emit_av(skt, et):
                        for cc in range(SQC):
                            nc.tensor.matmul(
                                po_h[:, cc, :],
                                v_sb[:, skt, 2 * p + h],
                                et[:, 512 * cc : 512 * (cc + 1)],
                                start=(skt == 0),
                                stop=(skt == ST - 1),
                            )

                    pend = []
                    for skt in range(ST):
                        psc = ps.tile([128, S], FP32, tag="big2", bufs=3)
                        for cc in range(SQC):
                            nc.tensor.matmul(
                                psc[:, 512 * cc : 512 * (cc + 1)],
                                kT[64 * h : 64 * (h + 1), 128 * skt : 128 * (skt + 1)],
                                qT[64 * h : 64 * (h + 1), 512 * cc : 512 * (cc + 1)],
                                start=True,
                                stop=True,
                            )
                        et = expp.tile([128, S], BF16, tag="exp", bufs=12)
                        nc.scalar.activation(et, psc, AF.Exp, scale=1.0 / 8.0)
                        if p == 0 and h == 0:
                            emit_v(skt)
                        if h == 1 and p + 1 < PAIRS and skt % 2 == 0:
                            j, cc = next_groups[skt // 2]
                            emit_qk_group(p + 1, *qk_tiles[p + 1], j, cc)
                        pend.append((skt, et))
                        if len(pend) > 4:
                            emit_av(*pend.pop(0))
                    for item in pend:
                        emit_av(*item)
                    # normalize: out / rowsum (rowsum is PSUM row 64).
                    # 1/r = exp(-ln(r)) on ScalarE: a 1-lane DVE reciprocal is
                    # ~8 cycles/element (4us per row on HW) while ACT runs
                    # 1 cycle/element and reads PSUM directly. The fp32
                    # reciprocal row is then broadcast across the 64 head-dim
                    # partitions with a stride-0 SBUF->SBUF DMA.
                    lnr = work.tile([1, SQC, 512], FP32, tag="lnr")
                    nc.scalar.activation(lnr, po_h[HD : HD + 1, :, :], AF.Ln)
                    recip = work.tile([1, SQC, 512], BF16, tag="recip")
                    nc.scalar.activation(recip, lnr, AF.Exp, scale=-1.0)
                    pou = work.tile([HD, SQC, 512], FP32, tag="pou")
                    nc.vector.tensor_copy(pou, po_h[0:HD, :, :])
                    for cc in range(SQC):
                        rbc = ps.tile(
                            [HD, 512], FP32, tag="acc", bufs=2, name=f"rbc{h}_{cc}"
                        )
                        nc.tensor.matmul(
                            rbc, ones_r64, recip[:, cc], start=True, stop=True
                        )
                        nc.vector.tensor_tensor(
                            outT[p][cc][64 * h : 64 * (h + 1), :],
                            pou[:, cc, :],
                            rbc,
                            ALU.mult,
                        )

            # ---------- output projection + residual ----------
            for t in range(ST):
                ph = ps.tile([128, C], FP32, tag="acc", bufs=2)
                for p in range(PAIRS):
                    nc.tensor.matmul(
                        ph,
                        outT[p][t // 4][:, 128 * (t % 4) : 128 * (t % 4 + 1)],
                        wout_sb[:, p],
                        start=(p == 0),
                        stop=(p == PAIRS - 1),
                    )
                yt = work.tile([128, C], FP32, tag="y")
                nc.vector.tensor_tensor(yt, ph, xn[:, t], ALU.add)
                if has_bout:
                    nc.vector.tensor_tensor(yt, yt, bout_bc, ALU.add)
                _dma(nc, y_d[128 * t : 128 * (t + 1), :], yt)

    return nc


_CACHE: dict = {}


def _get_program(has_bqk: bool = True, has_bv: bool = True, has_bout: bool = True) -> bass.Bass:
    key = (has_bqk, has_bv, has_bout)
    if key not in _CACHE:
        _install_bir_legalizer()
        nc = bass.Bass()
        _emit(nc, has_bqk, has_bv, has_bout)
        _CACHE[key] = nc
    return _CACHE[key]


def _get_runner(nc: bass.Bass):
    """Build the sharded PJRT executable once and cache it — bass2jax's
    run_bass_via_pjrt re-jits a fresh closure every call (~1.5 s of host-side
    retrace per invocation)."""
    if "runner" in _CACHE and _CACHE["runner"][0] is nc:
        return _CACHE["runner"][1]

    import jax
    from jax.sharding import Mesh, PartitionSpec
    from jax.experimental.shard_map import shard_map
    from concourse import bass2jax, mybir as _mb

    bass2jax.install_neuronx_cc_hook()
    partition_name = (
        nc.partition_id_tensor.name if nc.partition_id_tensor else None
    )
    in_names, out_names, out_avals, zero_outs = [], [], [], []
    for alloc in nc.m.functions[0].allocations:
        if not isinstance(alloc, _mb.MemoryLocationSet):
            continue
        name = alloc.memorylocations[0].name
        if alloc.kind == "ExternalInput":
            if name != partition_name:
                in_names.append(name)
        elif alloc.kind == "ExternalOutput":
            shape = tuple(alloc.tensor_shape)
            dtype = _mb.dt.np(alloc.dtype)
            out_names.append(name)
            out_avals.append(jax.core.ShapedArray(shape, dtype))
            zero_outs.append(np.zeros(shape, dtype))
    n_params = len(in_names)
    n_outs = len(out_avals)
    all_in_names = list(in_names) + list(out_names)
    if partition_name is not None:
        all_in_names.append(partition_name)
    donate = tuple(range(n_params, n_params + n_outs))

    def _body(*args):
        operands = list(args)
        if partition_name is not None:
            operands.append(bass2jax.partition_id_tensor())
        outs = bass2jax._bass_exec_p.bind(
            *operands,
            out_avals=tuple(out_avals),
            in_names=tuple(all_in_names),
            out_names=tuple(out_names),
            lowering_input_output_aliases=(),
            sim_require_finite=True,
            sim_require_nnan=True,
            nc=nc,
        )
        return tuple(outs)

    devices = jax.devices()[:N_CORES]
    mesh = Mesh(np.asarray(devices), ("core",))
    in_specs = (PartitionSpec("core"),) * (n_params + n_outs)
    out_specs = (PartitionSpec("core"),) * n_outs
    sharded = jax.jit(
        shard_map(
            _body, mesh=mesh, in_specs=in_specs, out_specs=out_specs,
            check_rep=False,
        ),
        donate_argnums=donate,
        keep_unused=True,
    )

    def run(in_maps):
        per_core = [[np.asarray(m[name]) for name in in_names] for m in in_maps]
        concat_in = [
            np.concatenate([per_core[c][i] for c in range(N_CORES)], axis=0)
            for i in range(n_params)
        ]
        concat_zeros = [
            np.zeros((N_CORES * z.shape[0], *z.shape[1:]), z.dtype)
            for z in zero_outs
        ]
        out_arrs = sharded(*concat_in, *concat_zeros)
        return [
            {
                name: np.asarray(out_arrs[i]).reshape(
                    N_CORES, *out_avals[i].shape
                )[c]
                for i, name in enumerate(out_names)
            }
            for c in range(N_CORES)
        ]

    _CACHE["runner"] = (nc, run)
    return run


def kernel(**inputs) -> np.ndarray:
    x = np.asarray(inputs["x"], dtype=np.float32)
    wqk, wv, wout = _pack_weights(
        np.asarray(inputs["w_qkv"], dtype=np.float32),
        np.asarray(inputs["w_out"], dtype=np.float32),
    )
    common = {
        "w_qk": wqk,
        "w_v": wv,
        "w_out": wout,
        "b_qkv": np.asarray(inputs["b_qkv"], dtype=np.float32),
        "b_out": np.asarray(inputs["b_out"], dtype=np.float32),
        "gn_scale": np.asarray(inputs["gn_scale"], dtype=np.float32),
        "gn_bias": np.asarray(inputs["gn_bias"], dtype=np.float32),
    }
    in_maps = [
        {"x": np.ascontiguousarray(x[i].reshape(S, C)), **common} for i in range(B)
    ]
    bq = common["b_qkv"]
    nc = _get_program(
        has_bqk=bool(np.any(bq.reshape(NH, 3, HD)[:, 0:2, :])),
        has_bv=bool(np.any(bq.reshape(NH, 3, HD)[:, 2, :])),
        has_bout=bool(np.any(common["b_out"])),
    )
    res = _get_runner(nc)(in_maps)
    y = np.stack([r["y"] for r in res], axis=0)
    return y.reshape(B, 32, 32, C).astype(np.float32)
